# revision 66
# baseline (speedup 1.0000x reference)
"""Trainium2 Bass kernel v3: batched single-head attention + residual + layernorm.

Per batch element b (one NeuronCore each, data-parallel over B=8):
    q = X@Wq+bq; k = X@Wk+bk; v = X@Wv+bv          [S=2048, K=64]
    attn = softmax(q @ k.T / 8, axis=-1)            [S, S]
    y = X + (attn @ v) @ Wo + bo                    [S, D=1024]
    out = layernorm(y) * gamma + beta

v3 design (v2 + timeline-driven scheduling/work changes):
  - Host passes X twice: X.T fp8e4m3 (projections) and X bf16 (residual);
    q/k/v weights packed into ONE [128,8,192] tensor whose rows are 1536B
    contiguous (single cheap DMA). Load plan: weights + two 1MB xt8 halves
    on SP/HWDGE; biases, wob8 and xb (8 chunks) on the Pool/SWDGE queue.
  - PE p-state warmup: ~88 dependency-free identity matmuls through the
    xt8-load window ramp the Tensor engine to full clock before the
    projections (cost model halves matmul speed for the first 3us of a
    busy streak).
  - Projections fp8 DoubleRow; block-0 k copy runs on ACT in parallel with
    DVE's q copy; scores bf16; exp on ACT 1024-wide into fp8 expT.
  - attn@v: fp8 DoubleRow with an extra v column of 1/64 accumulating
    sums/64 into psu row 64.
  - NO softmax division anywhere: layernorm is invariant to a positive
    per-row scale, so psy holds c*y with c = sums/32. av8 = uav/256 (fp8),
    the c-row joins avT8 row 32 (pairs with wob8 row 32 = bo), and X enters
    via diag(c) @ X - diag built per tile from a PE transpose of the c-row
    chunk into spare PSUM columns + a DVE identity-scale. rstd comes
    directly from the scaled stats: rstd' = (var' + 1e-3)^-0.5 via Ln/Exp.
  - LN engine split tuned from the simulated timeline: early tiles defer
    the j0 half to DVE one iteration later (ACT is exp-saturated); tiles
    >= 5 normalize fully on ACT; block-3 tiles alternate 2-bank PSUM tiles
    (1024-wide single-op ACT norms) with 1-bank pairs so two tiles stay in
    flight; block-3 diag tiles are prebuilt at iteration-12.
  - Score-pair emission: 8 pairs during projections (tgt 0), 6-pair seam
    prefill, then 2 per y-iteration, keeping ACT's exp stream saturated
    without parking PE's in-order queue on score PSUM slots.

gamma/beta are ones/zeros for this problem; applied on host if non-trivial.
"""

import numpy as np

B = 8
S = 2048
D = 1024
K = 64
EPS = 1e-5

NT = S // 128   # 16 s-tiles
NC_ = D // 128  # 8 d-chunks
NB = S // 512   # 4 query blocks

YS = 512.0      # psy = 512*y
AVS = 64.0      # av8 = 64*av ; v sums col = 1/64
WOS = 8.0       # wob8 = 8*Wo rows

_COMPILED = {}

# scheduling knobs (read at build time)
SCHED = {
    "emit_pos": "end",   # "mid": after stats j-loop; "end": end of iteration
    "early3": 0,          # iterations with 3 emissions (rest get 2)
    "av3_ti": 2,          # ti at which emit_av(3) fires (b==2)
    "prefill": 7,
    "outp_bufs": 4,
    "work_bufs": 4,
    "fillers": 88,
    "t_act": 5,
    "t11_psyt": True,
    "diag_pf_from": 3,
}


def _build_bass(act_norm_tiles=8, taps=False):
    import concourse.bacc as bacc
    import concourse.tile as tile
    from concourse import mybir
    from concourse.masks import make_identity

    f32 = mybir.dt.float32
    f32r = mybir.dt.float32r
    bf16 = mybir.dt.bfloat16
    f8 = mybir.dt.float8e4
    AF = mybir.ActivationFunctionType
    DR = mybir.MatmulPerfMode.DoubleRow

    nc = bacc.Bacc("TRN2", target_bir_lowering=False, debug=False)

    xb_dram = nc.dram_tensor("XB", [S, D], bf16, kind="ExternalInput")
    xt8_dram = nc.dram_tensor("XT8", [D, S], f8, kind="ExternalInput")
    # all projection weights packed host-side into one [128, 8, 192] tensor:
    # cols 0:64 q, 64:128 k, 128:192 v per d-chunk -> ONE 546ns DMA with
    # 1536B-contiguous rows instead of 4 small strided loads.
    wqkv8_dram = nc.dram_tensor("WQKV8", [128, NC_, 192], f8, kind="ExternalInput")
    bqk_dram = nc.dram_tensor("BQK", [128], f32, kind="ExternalInput")
    bv_dram = nc.dram_tensor("BV", [K], bf16, kind="ExternalInput")
    wob8_dram = nc.dram_tensor("WOB8", [33, 2, D], f8, kind="ExternalInput")
    out_dram = nc.dram_tensor("OUT", [S, D], bf16, kind="ExternalOutput")
    tap_handles = {}
    if taps:
        for name, shape, dt_ in [
            ("T_QK", [K, 2, S], mybir.dt.bfloat16),
            ("T_V", [128, NT, K + 1], mybir.dt.float8e4),
            ("T_EXP0", [128, NT, 512], mybir.dt.float8e4),
            ("T_AVT", [33, 2, S], mybir.dt.float8e4),
            ("T_RECB", [K, 512], mybir.dt.float32),
        ]:
            tap_handles[name] = nc.dram_tensor(name, shape, dt_, kind="ExternalOutput")

    with tile.TileContext(nc) as tc:
        with (
            tc.tile_pool(name="consts", bufs=1) as consts,
            tc.tile_pool(name="bigx", bufs=1) as bigx,
            tc.tile_pool(name="proj", bufs=1) as proj,
            tc.tile_pool(name="vtp", bufs=2) as vtp,
            tc.tile_pool(name="avn", bufs=SCHED.get("avn_bufs", 2)) as avn,
            tc.tile_pool(name="outp", bufs=SCHED["outp_bufs"]) as outp,
            tc.tile_pool(name="work", bufs=SCHED["work_bufs"]) as work,
            tc.tile_pool(name="expp", bufs=2) as expp,
            tc.tile_pool(name="psS", bufs=2, space="PSUM") as psS,
            tc.tile_pool(name="psU", bufs=1, space="PSUM") as psU,
        ):
            # Pre-place the act table that serves Exp+Ln+Identity+Copy so the
            # compiler's table-load pass doesn't flip-flop between the
            # exp-only and ln-only tables (1283ns per reload).
            nc.scalar.add_instruction(
                mybir.InstLoadActFuncSet(
                    name=nc.get_next_instruction_name(),
                    ins=[], outs=[], act_func_set_id=6,
                )
            )
            ident = consts.tile([128, 128], f32)
            make_identity(nc, ident)
            identb = consts.tile([128, 128], bf16)
            nc.gpsimd.tensor_copy(out=identb, in_=ident)
            # LN is invariant to a positive per-row scale, so psy holds
            # c*y with c = sums/32 (the softmax denominator, folded into the
            # diagonal of the residual matmul): rstd' = (var' + epsC)^-0.5
            # normalizes exactly. epsC approximates c^2*EPS; the mismatch is
            # <0.02% of var'.
            epsS_t = consts.tile([128, 1], f32)
            nc.vector.memset(epsS_t, 1e-3)
            ones_f = consts.tile([128, 512], f32)
            nc.gpsimd.memset(ones_f, 1.0)
            ones512r = consts.tile([1, 512], f32r)
            with nc.allow_low_precision(reason="ones constant to f32r"):
                nc.gpsimd.tensor_copy(out=ones512r, in_=ones_f[0:1, :])
            ones_row128 = consts.tile([1, 128], bf16)
            nc.gpsimd.memset(ones_row128, 1.0)

            # Load plan: SP/HWDGE queue carries the packed weights then xt8 in
            # two 1MB transfers (minimal desc-gen serialization -> first q
            # matmul ~4us earlier than 8 chunked loads). Pool/SWDGE carries
            # the small biases + wob8 + xb (desc-gen on the otherwise-idle
            # Pool engine, transfers interleave behind xt8 on the DMA device).
            xt8_sb = bigx.tile([128, NC_, S], f8)
            xt8_view = xt8_dram[:].rearrange("(c p) s -> p c s", p=128)
            xb_sb = bigx.tile([128, NT, D], bf16)
            xb_view = xb_dram[:].rearrange("(t p) d -> p t d", p=128)
            wqkv8 = consts.tile([128, NC_, 192], f8)
            bqk_row = consts.tile([1, 2, K], f32r)  # [q|k] bias as matmul lhsT
            bv_row8 = consts.tile([1, K], bf16)
            wob8 = consts.tile([33, 2, D], f8)
            # SP/HWDGE side: packed weights then the two xt8 halves
            nc.sync.dma_start(out=wqkv8, in_=wqkv8_dram[:])
            nc.sync.dma_start(out=xt8_sb[:, 0:4, :], in_=xt8_view[:, 0:4, :])
            nc.sync.dma_start(out=xt8_sb[:, 4:8, :], in_=xt8_view[:, 4:8, :])
            # Pool/SWDGE side: biases, wob8, then xb in 8 2-tile chunks
            # (small enough to not hog the DMA device at the avT8 seams)
            nc.gpsimd.dma_start(
                out=bqk_row,
                in_=bqk_dram[:].rearrange("(a j k) -> a j k", a=1, j=2).bitcast(f32r),
            )
            nc.gpsimd.dma_start(
                out=bv_row8, in_=bv_dram[:].rearrange("(a k) -> a k", a=1)
            )
            nc.gpsimd.dma_start(out=wob8, in_=wob8_dram[:])
            for h in range(8):
                nc.gpsimd.dma_start(
                    out=xb_sb[:, 2 * h : 2 * h + 2, :],
                    in_=xb_view[:, 2 * h : 2 * h + 2, :],
                )

            # PE p-state warmup: the cost model halves matmul throughput for
            # the first 3us of any busy streak (and quarters it at streak
            # start). A chain of dependency-free identity matmuls through the
            # xt8-load window ramps PE to full clock before the projections,
            # and keeps the streak alive until xt8's second half lands.
            warm = psS.tile([128, 2, 512], f32, tag="pss", name="warm")
            for _ in range(SCHED["fillers"]):
                nc.tensor.matmul(
                    warm[:, 0, 0:128], identb, identb,
                    start=True, stop=True,
                )

            qk2_sb = proj.tile([K, 2, S], bf16)  # [:,0,:] q, [:,1,:] k
            v_sb = proj.tile([128, NT, 80], f8)  # cols 0:64 v, col 64 = 1/64
            nc.gpsimd.memset(v_sb[:, :, K : K + 1], 1.0 / AVS)
            # avT8 rows 0:32 = uav/256 halves; row 32 ch0 = c-row (sums/32,
            # written per block from crowb), ch1 = 0 so its wob8 row is inert
            avT8 = proj.tile([33, 2, S], f8)
            nc.gpsimd.memset(avT8[32:33, 1, :], 0.0)

            exp_tiles = {}

            def emit_scores(tgt, pair_list):
                if tgt not in exp_tiles:
                    et = expp.tile([128, NT, 512], f8, tag="expT", name=f"expT{tgt}")
                    exp_tiles[tgt] = et
                et = exp_tiles[tgt]
                sqt = slice(tgt * 512, (tgt + 1) * 512)
                for p in pair_list:
                    pss = psS.tile([128, 2, 512], f32, tag="pss", name=f"pss{tgt}_{p}")
                    for j in range(2):
                        sk = 2 * p + j
                        nc.tensor.matmul(
                            pss[:, j, :],
                            qk2_sb[:, 1, sk * 128 : (sk + 1) * 128],
                            qk2_sb[:, 0, sqt],
                            start=True,
                            stop=True,
                        )
                    nc.scalar.activation(
                        out=et[:, 2 * p : 2 * p + 2, :], in_=pss[:],
                        func=AF.Exp, scale=0.125,
                    )

            pair_queue = [(tgt, p) for tgt in range(1, NB) for p in range(NT // 2)]

            def emit_next_pairs(n):
                for _ in range(n):
                    if pair_queue:
                        tgt, p = pair_queue.pop(0)
                        emit_scores(tgt, [p])

            # ---- phase 1: projections; block-0 scores piped in, block-1
            # scores emitted after (so expT(0) completes as early as possible
            # and phase 2 can start under the block-1 exp stream) ----
            with tc.tile_pool(name="psP", bufs=3, space="PSUM") as psP:
                for b in range(NB):
                    sq = slice(b * 512, (b + 1) * 512)
                    # q and k as separate 64-col chains, single-bank tiles
                    # (both land on partitions 0:64; DVE copies cannot cross
                    # partitions); biases folded in as ones-row matmuls.
                    # Block 0 interleaves the q/k chains across the two xt8
                    # DMA halves so PE starts as soon as half 1 lands.
                    psqk = [
                        psP.tile([K, 512], f32, tag=tg, bufs=1, name=f"ps{tg}")
                        for tg in ("psq", "psk")
                    ]
                    # bias matmul OPENS each accumulation group: it has no
                    # xt8 dependency, so it runs as soon as the biases land
                    # (~2.5us) instead of serializing after the DR chain --
                    # the chain then completes right at the last DR matmul
                    for j in range(2):
                        nc.tensor.matmul(
                            psqk[j],
                            bqk_row[:, j, :],
                            ones512r,
                            start=True,
                            stop=False,
                        )
                    for j, cp in [(j, cp) for j in range(2) for cp in range(4)]:
                        nc.tensor.matmul(
                            psqk[j],
                            wqkv8[:, 2 * cp : 2 * cp + 2, j * K : (j + 1) * K],
                            xt8_sb[:, 2 * cp : 2 * cp + 2, sq],
                            start=False,
                            stop=(cp == 3),
                            perf_mode=DR,
                        )
                    for j in range(2):
                        if b == 0 and j == 1:
                            # block 0 is latency-critical for the first exp:
                            # run the k copy on the still-idle ACT engine in
                            # parallel with DVE's q copy
                            nc.scalar.activation(
                                out=qk2_sb[:, j, sq], in_=psqk[j],
                                func=AF.Identity,
                            )
                        else:
                            nc.vector.tensor_copy(out=qk2_sb[:, j, sq], in_=psqk[j])
                    # scores for block 0 as its k-tiles become available;
                    # emitted before the v matmuls so the exp stream starts
                    # as early as possible; blocks 1-2 also pull 2 pairs from
                    # the global queue (ACT otherwise starves mid-projection)
                    emit_scores(0, range(b * 2, b * 2 + 2))
                    # v in natural [s, j] layout: xt8 chunks stationary,
                    # wv8 chunk moving; bias via a ones-row matmul
                    psv = psP.tile([128, 4, K], f32, tag="psv", bufs=1)
                    for ti in range(4):
                        t = b * 4 + ti
                        nc.tensor.matmul(
                            psv[:, ti, :],
                            ones_row128,
                            bv_row8,
                            start=True,
                            stop=False,
                        )
                        for c in range(NC_):
                            nc.tensor.matmul(
                                psv[:, ti, :],
                                xt8_sb[:, c, t * 128 : (t + 1) * 128],
                                wqkv8[:, c, 128:192],
                                start=False,
                                stop=(c == NC_ - 1),
                            )
                    nc.vector.tensor_copy(
                        out=v_sb[:, b * 4 : (b + 1) * 4, 0:K], in_=psv
                    )

            # ---- phase 2 ----
            out_view = out_dram[:].rearrange("(t p) d -> p t d", p=128)

            # c-row staging: row 0 carries c = sums/32 per block; rows 1:31
            # stay zero so the per-tile PE transpose of [32,128] chunks reads
            # defined data
            crowb = proj.tile([32, S], f32)
            nc.gpsimd.memset(crowb[:, :], 0.0)

            diags = {}
            psu_by_block = {}

            def emit_block_diags(b):
                # prebuild block b's four diag(c) tiles (transposes into
                # psu_b's spare columns). The per-tile diag otherwise queues
                # behind DVE's stats backlog and gates every y matmul.
                # Called at iteration-(4b) top so the PE park on the c-row
                # cast sits AFTER the previous block's matmuls.
                psu = psu_by_block.pop(b)
                for ti in range(4):
                    t = 4 * b + ti
                    cs = 384 + 32 * ti
                    nc.tensor.transpose(
                        out=psu[:, cs : cs + 32],
                        in_=crowb[:, t * 128 : (t + 1) * 128],
                        identity=ident[0:32, 0:32],
                    )
                    dt_ = work.tile(
                        [128, 128], bf16, tag="diag", name=f"diag{t}"
                    )
                    nc.vector.tensor_scalar(
                        out=dt_, in0=identb, scalar1=psu[:, cs : cs + 1],
                        scalar2=None, op0=mybir.AluOpType.mult,
                    )
                    diags[t] = dt_

            def emit_av(b, splits=1):
                """uav -> av8 = uav/256 cast + c-row = sums/32 for block b,
                then ALL FOUR of the block's diag(c) tiles. Prebuilding the
                diagonals at the seam keeps them out of the per-tile DVE
                queue, whose stats backlog otherwise gates the y matmuls.
                No softmax division: LN's scale-invariance absorbs the
                denominator via the per-row c in the residual diagonal."""
                expT = exp_tiles.pop(b)
                psu = psU.tile([128, 512], f32, tag="psu", name=f"psu{b}")
                for tp in range(NT // 2):
                    nc.tensor.matmul(
                        psu[0 : K + 1, :],
                        v_sb[:, 2 * tp : 2 * tp + 2, 0 : K + 1],
                        expT[:, 2 * tp : 2 * tp + 2, :],
                        start=(tp == 0),
                        stop=(tp == NT // 2 - 1),
                        perf_mode=DR,
                    )
                av8 = avn.tile([K, 512], f8, tag="av8")
                sq = slice(b * 512, (b + 1) * 512)
                with nc.allow_low_precision(reason="uav cast to f8"):
                    nc.vector.tensor_scalar(
                        out=av8, in0=psu[0:K, :], scalar1=1.0 / 256.0,
                        scalar2=None, op0=mybir.AluOpType.mult,
                    )
                nc.vector.tensor_scalar(
                    out=crowb[0:1, sq], in0=psu[K : K + 1, :], scalar1=2.0,
                    scalar2=None, op0=mybir.AluOpType.mult,
                )
                nc.sync.dma_start(out=avT8[0:32, 0, sq], in_=av8[0:32, :])
                nc.sync.dma_start(out=avT8[0:32, 1, sq], in_=av8[32:K, :])
                # c-row into avT8 on Pool (SBUF->SBUF bf16->f8 convert)
                with nc.allow_low_precision(reason="c-row cast to f8"):
                    nc.gpsimd.tensor_copy(
                        out=avT8[32:33, 0, sq], in_=crowb[0:1, sq]
                    )
                psu_by_block[b] = psu
                if taps and b == 0:
                    nc.gpsimd.dma_start(out=tap_handles["T_RECB"][:], in_=crowb[0:8, 0:512])

            if taps:
                nc.gpsimd.dma_start(out=tap_handles["T_QK"][:], in_=qk2_sb[:])
                nc.gpsimd.dma_start(out=tap_handles["T_V"][:], in_=v_sb[:, :, 0 : K + 1])
            # Software-pipelined LN: at iteration t, the j=1 half is normalized
            # on ACT (same engine as rstd, no cross-engine wait); the j=0 half
            # of iteration t-1 is normalized on DVE using the then-ready rstd,
            # so the in-order DVE queue never waits on ACT.
            #
            # Remaining score-pairs (blocks 1-3) are fed from a global queue,
            # 2 per tile iteration AFTER that tile's y-work, so the in-order
            # PE queue never parks y matmuls behind exp-paced score matmuls.
            with tc.tile_pool(name="psY", bufs=3, space="PSUM") as psY:
                if taps:
                    nc.gpsimd.dma_start(
                        out=tap_handles["T_EXP0"][:], in_=exp_tiles[0][:]
                    )
                emit_av(0)
                emit_next_pairs(SCHED["prefill"])  # seam pre-fill
                prev = None  # (psy0, out_sb, mv, rstd, t)
                for b in range(NB):
                    if taps and b == NB - 1:
                        nc.gpsimd.dma_start(out=tap_handles["T_AVT"][:], in_=avT8[:])
                    for ti in range(4):
                        t = b * 4 + ti
                        if ti == 0 and b >= SCHED["diag_pf_from"]:
                            emit_block_diags(b)
                        out_sb = outp.tile([128, D], bf16, tag="o")
                        psy = [None, None]
                        stats = work.tile([128, 2, 6], f32, tag="stats")
                        # block 3: the score-psum pool is free; use its 2-bank
                        # tiles for y so stats/norm run 1024-wide and the LN
                        # pipeline gets extra depth
                        psyt = None
                        if (b == NB - 1 and ti % 2 == 0) or (
                            t == 11 and SCHED["t11_psyt"]
                        ):
                            psyt = psS.tile(
                                [128, 2, 512], f32, tag="pss", name=f"psy2_{t}"
                            )
                            psy[0] = psyt[:, 0, :]
                            psy[1] = psyt[:, 1, :]
                        else:
                            psy[0] = psY.tile([128, 512], f32, tag="ps", name=f"psy0_{t}")
                            psy[1] = psY.tile([128, 512], f32, tag="ps", name=f"psy1_{t}")
                        if t in diags:
                            diag_t = diags.pop(t)
                        else:
                            # per-row diag(c): transpose this tile's c chunk
                            # into spare psy columns (overwritten by the y
                            # matmuls right after diag is built)
                            nc.tensor.transpose(
                                out=psy[0][:, 0:32],
                                in_=crowb[:, t * 128 : (t + 1) * 128],
                                identity=ident[0:32, 0:32],
                            )
                            diag_t = work.tile([128, 128], bf16, tag="diag")
                            nc.vector.tensor_scalar(
                                out=diag_t, in0=identb, scalar1=psy[0][:, 0:1],
                                scalar2=None, op0=mybir.AluOpType.mult,
                            )
                        for j in range(2):
                            psy_j = psy[j]
                            nc.tensor.matmul(
                                psy_j,
                                avT8[:, :, t * 128 : (t + 1) * 128],
                                wob8[:, :, j * 512 : (j + 1) * 512],
                                start=True,
                                stop=False,
                                perf_mode=DR,
                            )
                            nc.tensor.matmul(
                                psy_j,
                                diag_t,
                                xb_sb[:, t, j * 512 : (j + 1) * 512],
                                start=False,
                                stop=True,
                            )
                            nc.vector.bn_stats(out=stats[:, j, :], in_=psy_j)
                        n_emit = 3 if t < SCHED["early3"] else 2
                        if SCHED["emit_pos"] == "mid":
                            emit_next_pairs(n_emit)
                        mv = work.tile([128, 2], f32, tag="mv")
                        nc.vector.bn_aggr(out=mv, in_=stats)
                        # ACT-local chain (no DVE hop): mneg, then
                        # rstd = (var'+epsC)^-0.5 = exp(-0.5*ln(var'+epsC))
                        mneg = work.tile([128, 1], f32, tag="mneg")
                        nc.scalar.mul(mneg, mv[:, 0:1], -1.0)
                        lnv = work.tile([128, 1], f32, tag="lnv")
                        nc.scalar.activation(
                            out=lnv, in_=mv[:, 1:2], func=AF.Ln,
                            bias=epsS_t, scale=1.0,
                        )
                        rstd = work.tile([128, 1], f32, tag="rstd")
                        nc.scalar.activation(
                            out=rstd, in_=lnv, func=AF.Exp, scale=-0.5,
                        )
                        # nm = -mu*rstd, on ACT so the chain stays ACT-local
                        nm = work.tile([128, 1], f32, tag="nm")
                        nc.scalar.activation(
                            out=nm, in_=mneg, func=AF.Copy, scale=rstd,
                        )
                        if psyt is None:
                            nc.scalar.activation(
                                out=out_sb[:, 512:1024], in_=psy[1],
                                func=AF.Identity, bias=nm, scale=rstd,
                            )
                        if prev is not None:
                            p_psy0, p_out, p_mv, p_rstd, p_t = prev
                            nc.vector.tensor_scalar(
                                out=p_out[:, 0:512], in0=p_psy0,
                                scalar1=p_mv[:, 0:1], scalar2=p_rstd,
                                op0=mybir.AluOpType.subtract,
                                op1=mybir.AluOpType.mult,
                            )
                            nc.sync.dma_start(
                                out=out_view[:, p_t, :], in_=p_out
                            )
                            prev = None
                        if psyt is not None:
                            # block-3 + t=11 tiles: exps are done, ACT has
                            # slack and DVE (stats-bound) is critical ->
                            # normalize 1024-wide in one ACT op
                            nc.scalar.activation(
                                out=out_sb[:].rearrange(
                                    "p (j d) -> p j d", j=2
                                ),
                                in_=psyt,
                                func=AF.Identity, bias=nm, scale=rstd,
                            )
                            nc.sync.dma_start(
                                out=out_view[:, t, :], in_=out_sb
                            )
                        elif t >= 12:
                            # non-psyt block-3 tiles: j1 was normalized on ACT
                            # above (stored immediately as its own half so the
                            # DMA overlaps DVE's j0 norm); j0 on DVE here
                            nc.sync.dma_start(
                                out=out_view[:, t, 512:1024],
                                in_=out_sb[:, 512:1024],
                            )
                            nc.vector.tensor_scalar(
                                out=out_sb[:, 0:512], in0=psy[0],
                                scalar1=mv[:, 0:1], scalar2=rstd,
                                op0=mybir.AluOpType.subtract,
                                op1=mybir.AluOpType.mult,
                            )
                            nc.sync.dma_start(
                                out=out_view[:, t, 0:512], in_=out_sb[:, 0:512]
                            )
                        elif t >= SCHED["t_act"]:
                            # late non-psyt tiles: exp stream has drained, so
                            # ACT takes the j0 half too (DVE keeps only stats)
                            nc.scalar.activation(
                                out=out_sb[:, 0:512], in_=psy[0],
                                func=AF.Identity, bias=nm, scale=rstd,
                            )
                            nc.sync.dma_start(
                                out=out_view[:, t, :], in_=out_sb
                            )
                        else:
                            prev = (psy[0], out_sb, mv, rstd, t)
                        if ti == (SCHED["av3_ti"] if b == 2 else 2) and b + 1 < NB:
                            emit_av(b + 1)
                        if SCHED["emit_pos"] == "end":
                            emit_next_pairs(n_emit)
                # tail: finish any pending deferred tile
                if prev is not None:
                    p_psy0, p_out, p_mv, p_rstd, p_t = prev
                    nc.vector.tensor_scalar(
                        out=p_out[:, 0:512], in0=p_psy0,
                        scalar1=p_mv[:, 0:1], scalar2=p_rstd,
                        op0=mybir.AluOpType.subtract,
                        op1=mybir.AluOpType.mult,
                    )
                    nc.sync.dma_start(out=out_view[:, p_t, :], in_=p_out)

    nc.compile()
    return nc


def _get_compiled():
    if "nc" not in _COMPILED:
        _COMPILED["nc"] = _build_bass()
    return _COMPILED["nc"]


def _host_inputs(X, Wq, bq, Wk, bk, Wv, bv, Wo, bo):
    import ml_dtypes

    f8 = ml_dtypes.float8_e4m3
    bf = ml_dtypes.bfloat16
    f32 = np.float32

    # [D, 192] = [q|k|v] weights -> [128, 8, 192] with d = c*128 + p
    wqkv = np.concatenate([Wq, Wk, Wv], axis=1).astype(f32)
    wqkv8 = np.ascontiguousarray(
        wqkv.reshape(NC_, 128, 192).transpose(1, 0, 2)
    ).astype(f8)
    bqk = np.concatenate([bq, bk]).astype(f32)
    wob8 = np.zeros((33, 2, D), dtype=f8)
    wo8 = (Wo.astype(f32) * WOS).astype(f8)
    wob8[:32, 0, :] = wo8[0:32]
    wob8[:32, 1, :] = wo8[32:64]
    # row 32 pairs with the c-row (sums/32) in avT8: contributes c*bo
    wob8[32, 0, :] = bo.astype(f32).astype(f8)

    common = {
        "WQKV8": wqkv8,
        "BQK": bqk,
        "BV": bv.astype(f32).astype(bf),
        "WOB8": wob8,
    }
    per_core = []
    for i in range(X.shape[0]):
        Xi = np.ascontiguousarray(X[i], dtype=f32)
        per_core.append(
            {
                "XB": Xi.astype(bf),
                "XT8": np.ascontiguousarray(Xi.T).astype(f8),
                **common,
            }
        )
    return per_core


def kernel(X, Wq, bq, Wk, bk, Wv, bv, Wo, bo, gamma, beta):
    from concourse.bass_utils import run_bass_kernel_spmd

    X = np.asarray(X, dtype=np.float32)
    gamma_np = np.asarray(gamma, dtype=np.float32)
    beta_np = np.asarray(beta, dtype=np.float32)

    nc = _get_compiled()
    in_maps = _host_inputs(
        X,
        np.asarray(Wq), np.asarray(bq), np.asarray(Wk), np.asarray(bk),
        np.asarray(Wv), np.asarray(bv), np.asarray(Wo), np.asarray(bo),
    )
    res = run_bass_kernel_spmd(nc, in_maps, core_ids=list(range(B)))
    out = np.stack(
        [np.asarray(res.results[i]["OUT"]).astype(np.float32) for i in range(B)],
        axis=0,
    )
    if not (np.all(gamma_np == 1.0) and np.all(beta_np == 0.0)):
        out = out * gamma_np + beta_np
    return out.astype(np.float32)



# revision 68
# speedup vs baseline: 1.0051x; 1.0051x over previous
"""Trainium2 Bass kernel v3: batched single-head attention + residual + layernorm.

Per batch element b (one NeuronCore each, data-parallel over B=8):
    q = X@Wq+bq; k = X@Wk+bk; v = X@Wv+bv          [S=2048, K=64]
    attn = softmax(q @ k.T / 8, axis=-1)            [S, S]
    y = X + (attn @ v) @ Wo + bo                    [S, D=1024]
    out = layernorm(y) * gamma + beta

v3 design (v2 + timeline-driven scheduling/work changes):
  - Host passes X twice: X.T fp8e4m3 (projections) and X bf16 (residual);
    q/k/v weights packed into ONE [128,8,192] tensor whose rows are 1536B
    contiguous (single cheap DMA). Load plan: weights + two 1MB xt8 halves
    on SP/HWDGE; biases, wob8 and xb (8 chunks) on the Pool/SWDGE queue.
  - PE p-state warmup: ~88 dependency-free identity matmuls through the
    xt8-load window ramp the Tensor engine to full clock before the
    projections (cost model halves matmul speed for the first 3us of a
    busy streak).
  - Projections fp8 DoubleRow; block-0 k copy runs on ACT in parallel with
    DVE's q copy; scores bf16; exp on ACT 1024-wide into fp8 expT.
  - attn@v: fp8 DoubleRow with an extra v column of 1/64 accumulating
    sums/64 into psu row 64.
  - NO softmax division anywhere: layernorm is invariant to a positive
    per-row scale, so psy holds c*y with c = sums/32. av8 = uav/256 (fp8),
    the c-row joins avT8 row 32 (pairs with wob8 row 32 = bo), and X enters
    via diag(c) @ X - diag built per tile from a PE transpose of the c-row
    chunk into spare PSUM columns + a DVE identity-scale. rstd comes
    directly from the scaled stats: rstd' = (var' + 1e-3)^-0.5 via Ln/Exp.
  - LN engine split tuned from the simulated timeline: early tiles defer
    the j0 half to DVE one iteration later (ACT is exp-saturated); tiles
    >= 5 normalize fully on ACT; block-3 tiles alternate 2-bank PSUM tiles
    (1024-wide single-op ACT norms) with 1-bank pairs so two tiles stay in
    flight; block-3 diag tiles are prebuilt at iteration-12.
  - Score-pair emission: 8 pairs during projections (tgt 0), 6-pair seam
    prefill, then 2 per y-iteration, keeping ACT's exp stream saturated
    without parking PE's in-order queue on score PSUM slots.

gamma/beta are ones/zeros for this problem; applied on host if non-trivial.
"""

import numpy as np

B = 8
S = 2048
D = 1024
K = 64
EPS = 1e-5

NT = S // 128   # 16 s-tiles
NC_ = D // 128  # 8 d-chunks
NB = S // 512   # 4 query blocks

YS = 512.0      # psy = 512*y
AVS = 64.0      # av8 = 64*av ; v sums col = 1/64
WOS = 8.0       # wob8 = 8*Wo rows

_COMPILED = {}

# scheduling knobs (read at build time)
SCHED = {
    "emit_pos": "end",   # "mid": after stats j-loop; "end": end of iteration
    "early3": 0,          # iterations with 3 emissions (rest get 2)
    "av3_ti": 2,          # ti at which emit_av(3) fires (b==2)
    "prefill": 7,
    "outp_bufs": 4,
    "work_bufs": 4,
    "fillers": 88,
    "t_act": 5,
    "t11_psyt": True,
    "diag_pf_from": 3,
}


def _build_bass(act_norm_tiles=8, taps=False):
    import concourse.bacc as bacc
    import concourse.tile as tile
    from concourse import mybir
    from concourse.masks import make_identity

    f32 = mybir.dt.float32
    f32r = mybir.dt.float32r
    bf16 = mybir.dt.bfloat16
    f8 = mybir.dt.float8e4
    AF = mybir.ActivationFunctionType
    DR = mybir.MatmulPerfMode.DoubleRow

    nc = bacc.Bacc("TRN2", target_bir_lowering=False, debug=False)

    xb_dram = nc.dram_tensor("XB", [S, D], bf16, kind="ExternalInput")
    xt8_dram = nc.dram_tensor("XT8", [D, S], f8, kind="ExternalInput")
    # all projection weights packed host-side into one [128, 8, 192] tensor:
    # cols 0:64 q, 64:128 k, 128:192 v per d-chunk -> ONE 546ns DMA with
    # 1536B-contiguous rows instead of 4 small strided loads.
    wqkv8_dram = nc.dram_tensor("WQKV8", [128, NC_, 192], f8, kind="ExternalInput")
    bqk_dram = nc.dram_tensor("BQK", [128], f32, kind="ExternalInput")
    bv_dram = nc.dram_tensor("BV", [K], bf16, kind="ExternalInput")
    wob8_dram = nc.dram_tensor("WOB8", [33, 2, D], f8, kind="ExternalInput")
    out_dram = nc.dram_tensor("OUT", [S, D], bf16, kind="ExternalOutput")
    tap_handles = {}
    if taps:
        for name, shape, dt_ in [
            ("T_QK", [K, 2, S], mybir.dt.bfloat16),
            ("T_V", [128, NT, K + 1], mybir.dt.float8e4),
            ("T_EXP0", [128, NT, 512], mybir.dt.float8e4),
            ("T_AVT", [33, 2, S], mybir.dt.float8e4),
            ("T_RECB", [K, 512], mybir.dt.float32),
        ]:
            tap_handles[name] = nc.dram_tensor(name, shape, dt_, kind="ExternalOutput")

    with tile.TileContext(nc) as tc:
        with (
            tc.tile_pool(name="consts", bufs=1) as consts,
            tc.tile_pool(name="bigx", bufs=1) as bigx,
            tc.tile_pool(name="proj", bufs=1) as proj,
            tc.tile_pool(name="vtp", bufs=2) as vtp,
            tc.tile_pool(name="avn", bufs=SCHED.get("avn_bufs", 2)) as avn,
            tc.tile_pool(name="outp", bufs=SCHED["outp_bufs"]) as outp,
            tc.tile_pool(name="work", bufs=SCHED["work_bufs"]) as work,
            tc.tile_pool(name="expp", bufs=2) as expp,
            tc.tile_pool(name="psS", bufs=2, space="PSUM") as psS,
            tc.tile_pool(name="psU", bufs=1, space="PSUM") as psU,
        ):
            # Pre-place the act table that serves Exp+Ln+Identity+Copy so the
            # compiler's table-load pass doesn't flip-flop between the
            # exp-only and ln-only tables (1283ns per reload).
            nc.scalar.add_instruction(
                mybir.InstLoadActFuncSet(
                    name=nc.get_next_instruction_name(),
                    ins=[], outs=[], act_func_set_id=6,
                )
            )
            ident = consts.tile([128, 128], f32)
            make_identity(nc, ident)
            identb = consts.tile([128, 128], bf16)
            nc.gpsimd.tensor_copy(out=identb, in_=ident)
            # LN is invariant to a positive per-row scale, so psy holds
            # c*y with c = sums/32 (the softmax denominator, folded into the
            # diagonal of the residual matmul): rstd' = (var' + epsC)^-0.5
            # normalizes exactly. epsC approximates c^2*EPS; the mismatch is
            # <0.02% of var'.
            epsS_t = consts.tile([128, 1], f32)
            nc.vector.memset(epsS_t, 1e-3)
            ones_f = consts.tile([128, 512], f32)
            nc.gpsimd.memset(ones_f, 1.0)
            ones512r = consts.tile([1, 512], f32r)
            with nc.allow_low_precision(reason="ones constant to f32r"):
                nc.gpsimd.tensor_copy(out=ones512r, in_=ones_f[0:1, :])
            ones_row128 = consts.tile([1, 128], bf16)
            nc.gpsimd.memset(ones_row128, 1.0)

            # Load plan: SP/HWDGE queue carries the packed weights then xt8 in
            # two 1MB transfers (minimal desc-gen serialization -> first q
            # matmul ~4us earlier than 8 chunked loads). Pool/SWDGE carries
            # the small biases + wob8 + xb (desc-gen on the otherwise-idle
            # Pool engine, transfers interleave behind xt8 on the DMA device).
            xt8_sb = bigx.tile([128, NC_, S], f8)
            xt8_view = xt8_dram[:].rearrange("(c p) s -> p c s", p=128)
            xb_sb = bigx.tile([128, NT, D], bf16)
            xb_view = xb_dram[:].rearrange("(t p) d -> p t d", p=128)
            wqkv8 = consts.tile([128, NC_, 192], f8)
            bqk_row = consts.tile([1, 2, K], f32r)  # [q|k] bias as matmul lhsT
            bv_row8 = consts.tile([1, K], bf16)
            wob8 = consts.tile([33, 2, D], f8)
            # SP/HWDGE side: packed weights then the two xt8 halves
            nc.sync.dma_start(out=wqkv8, in_=wqkv8_dram[:])
            nc.sync.dma_start(out=xt8_sb[:, 0:4, :], in_=xt8_view[:, 0:4, :])
            nc.sync.dma_start(out=xt8_sb[:, 4:8, :], in_=xt8_view[:, 4:8, :])
            # Pool/SWDGE side: biases, wob8, then xb in 8 2-tile chunks
            # (small enough to not hog the DMA device at the avT8 seams)
            nc.gpsimd.dma_start(
                out=bqk_row,
                in_=bqk_dram[:].rearrange("(a j k) -> a j k", a=1, j=2).bitcast(f32r),
            )
            nc.gpsimd.dma_start(
                out=bv_row8, in_=bv_dram[:].rearrange("(a k) -> a k", a=1)
            )
            nc.gpsimd.dma_start(out=wob8, in_=wob8_dram[:])
            for h in range(8):
                nc.gpsimd.dma_start(
                    out=xb_sb[:, 2 * h : 2 * h + 2, :],
                    in_=xb_view[:, 2 * h : 2 * h + 2, :],
                )

            # PE p-state warmup: the cost model halves matmul throughput for
            # the first 3us of any busy streak (and quarters it at streak
            # start). A chain of dependency-free identity matmuls through the
            # xt8-load window ramps PE to full clock before the projections,
            # and keeps the streak alive until xt8's second half lands.
            warm = psS.tile([128, 2, 512], f32, tag="pss", name="warm")
            for _ in range(SCHED["fillers"]):
                nc.tensor.matmul(
                    warm[:, 0, 0:128], identb, identb,
                    start=True, stop=True,
                )

            qk2_sb = proj.tile([K, 2, S], bf16)  # [:,0,:] q, [:,1,:] k
            v_sb = proj.tile([128, NT, 80], f8)  # cols 0:64 v, col 64 = 1/64
            nc.gpsimd.memset(v_sb[:, :, K : K + 1], 1.0 / AVS)
            # avT8 rows 0:32 = uav/256 halves; row 32 ch0 = c-row (sums/32,
            # written per block from crowb), ch1 = 0 so its wob8 row is inert
            avT8 = proj.tile([33, 2, S], f8)
            nc.gpsimd.memset(avT8[32:33, 1, :], 0.0)

            exp_tiles = {}

            def emit_scores(tgt, pair_list):
                if tgt not in exp_tiles:
                    et = expp.tile([128, NT, 512], f8, tag="expT", name=f"expT{tgt}")
                    exp_tiles[tgt] = et
                et = exp_tiles[tgt]
                sqt = slice(tgt * 512, (tgt + 1) * 512)
                for p in pair_list:
                    pss = psS.tile([128, 2, 512], f32, tag="pss", name=f"pss{tgt}_{p}")
                    for j in range(2):
                        sk = 2 * p + j
                        nc.tensor.matmul(
                            pss[:, j, :],
                            qk2_sb[:, 1, sk * 128 : (sk + 1) * 128],
                            qk2_sb[:, 0, sqt],
                            start=True,
                            stop=True,
                        )
                    nc.scalar.activation(
                        out=et[:, 2 * p : 2 * p + 2, :], in_=pss[:],
                        func=AF.Exp, scale=0.125,
                    )

            pair_queue = [(tgt, p) for tgt in range(1, NB) for p in range(NT // 2)]

            def emit_next_pairs(n):
                for _ in range(n):
                    if pair_queue:
                        tgt, p = pair_queue.pop(0)
                        emit_scores(tgt, [p])

            # ---- phase 1: projections; block-0 scores piped in, block-1
            # scores emitted after (so expT(0) completes as early as possible
            # and phase 2 can start under the block-1 exp stream) ----
            with tc.tile_pool(name="psP", bufs=3, space="PSUM") as psP:
                for b in range(NB):
                    sq = slice(b * 512, (b + 1) * 512)
                    # q and k as separate 64-col chains, single-bank tiles
                    # (both land on partitions 0:64; DVE copies cannot cross
                    # partitions); biases folded in as ones-row matmuls.
                    # Block 0 interleaves the q/k chains across the two xt8
                    # DMA halves so PE starts as soon as half 1 lands.
                    psqk = [
                        psP.tile([K, 512], f32, tag=tg, bufs=1, name=f"ps{tg}")
                        for tg in ("psq", "psk")
                    ]
                    for j, cp in [(j, cp) for j in range(2) for cp in range(4)]:
                        nc.tensor.matmul(
                            psqk[j],
                            wqkv8[:, 2 * cp : 2 * cp + 2, j * K : (j + 1) * K],
                            xt8_sb[:, 2 * cp : 2 * cp + 2, sq],
                            start=(cp == 0),
                            stop=False,
                            perf_mode=DR,
                        )
                    for j in range(2):
                        nc.tensor.matmul(
                            psqk[j],
                            bqk_row[:, j, :],
                            ones512r,
                            start=False,
                            stop=True,
                        )
                        if b == 0 and j == 1:
                            # block 0 is latency-critical for the first exp:
                            # run the k copy on the still-idle ACT engine in
                            # parallel with DVE's q copy
                            nc.scalar.activation(
                                out=qk2_sb[:, j, sq], in_=psqk[j],
                                func=AF.Identity,
                            )
                        else:
                            nc.vector.tensor_copy(out=qk2_sb[:, j, sq], in_=psqk[j])
                    # scores for block 0 as its k-tiles become available;
                    # emitted before the v matmuls so the exp stream starts
                    # as early as possible; blocks 1-2 also pull 2 pairs from
                    # the global queue (ACT otherwise starves mid-projection)
                    emit_scores(0, range(b * 2, b * 2 + 2))
                    # v in natural [s, j] layout: xt8 chunks stationary,
                    # wv8 chunk moving; bias via a ones-row matmul
                    psv = psP.tile([128, 4, K], f32, tag="psv", bufs=1)
                    for ti in range(4):
                        t = b * 4 + ti
                        for c in range(NC_):
                            nc.tensor.matmul(
                                psv[:, ti, :],
                                xt8_sb[:, c, t * 128 : (t + 1) * 128],
                                wqkv8[:, c, 128:192],
                                start=(c == 0),
                                stop=False,
                            )
                        nc.tensor.matmul(
                            psv[:, ti, :],
                            ones_row128,
                            bv_row8,
                            start=False,
                            stop=True,
                        )
                    nc.vector.tensor_copy(
                        out=v_sb[:, b * 4 : (b + 1) * 4, 0:K], in_=psv
                    )

            # ---- phase 2 ----
            out_view = out_dram[:].rearrange("(t p) d -> p t d", p=128)

            # c-row staging: row 0 carries c = sums/32 per block; rows 1:31
            # stay zero so the per-tile PE transpose of [32,128] chunks reads
            # defined data
            crowb = proj.tile([32, S], f32)
            nc.gpsimd.memset(crowb[:, :], 0.0)

            diags = {}
            psu_by_block = {}

            def emit_block_diags(b):
                # prebuild block b's four diag(c) tiles (transposes into
                # psu_b's spare columns). The per-tile diag otherwise queues
                # behind DVE's stats backlog and gates every y matmul.
                # Called at iteration-(4b) top so the PE park on the c-row
                # cast sits AFTER the previous block's matmuls.
                psu = psu_by_block.pop(b)
                for ti in range(4):
                    t = 4 * b + ti
                    cs = 384 + 32 * ti
                    nc.tensor.transpose(
                        out=psu[:, cs : cs + 32],
                        in_=crowb[:, t * 128 : (t + 1) * 128],
                        identity=ident[0:32, 0:32],
                    )
                    dt_ = work.tile(
                        [128, 128], bf16, tag="diag", name=f"diag{t}"
                    )
                    nc.vector.tensor_scalar(
                        out=dt_, in0=identb, scalar1=psu[:, cs : cs + 1],
                        scalar2=None, op0=mybir.AluOpType.mult,
                    )
                    diags[t] = dt_

            def emit_av(b, splits=1):
                """uav -> av8 = uav/256 cast + c-row = sums/32 for block b,
                then ALL FOUR of the block's diag(c) tiles. Prebuilding the
                diagonals at the seam keeps them out of the per-tile DVE
                queue, whose stats backlog otherwise gates the y matmuls.
                No softmax division: LN's scale-invariance absorbs the
                denominator via the per-row c in the residual diagonal."""
                expT = exp_tiles.pop(b)
                psu = psU.tile([128, 512], f32, tag="psu", name=f"psu{b}")
                for tp in range(NT // 2):
                    nc.tensor.matmul(
                        psu[0 : K + 1, :],
                        v_sb[:, 2 * tp : 2 * tp + 2, 0 : K + 1],
                        expT[:, 2 * tp : 2 * tp + 2, :],
                        start=(tp == 0),
                        stop=(tp == NT // 2 - 1),
                        perf_mode=DR,
                    )
                av8 = avn.tile([K, 512], f8, tag="av8")
                sq = slice(b * 512, (b + 1) * 512)
                with nc.allow_low_precision(reason="uav cast to f8"):
                    nc.vector.tensor_scalar(
                        out=av8, in0=psu[0:K, :], scalar1=1.0 / 256.0,
                        scalar2=None, op0=mybir.AluOpType.mult,
                    )
                nc.vector.tensor_scalar(
                    out=crowb[0:1, sq], in0=psu[K : K + 1, :], scalar1=2.0,
                    scalar2=None, op0=mybir.AluOpType.mult,
                )
                nc.sync.dma_start(out=avT8[0:32, 0, sq], in_=av8[0:32, :])
                nc.sync.dma_start(out=avT8[0:32, 1, sq], in_=av8[32:K, :])
                # c-row into avT8 on Pool (SBUF->SBUF bf16->f8 convert)
                with nc.allow_low_precision(reason="c-row cast to f8"):
                    nc.gpsimd.tensor_copy(
                        out=avT8[32:33, 0, sq], in_=crowb[0:1, sq]
                    )
                psu_by_block[b] = psu
                if taps and b == 0:
                    nc.gpsimd.dma_start(out=tap_handles["T_RECB"][:], in_=crowb[0:8, 0:512])

            if taps:
                nc.gpsimd.dma_start(out=tap_handles["T_QK"][:], in_=qk2_sb[:])
                nc.gpsimd.dma_start(out=tap_handles["T_V"][:], in_=v_sb[:, :, 0 : K + 1])
            # Software-pipelined LN: at iteration t, the j=1 half is normalized
            # on ACT (same engine as rstd, no cross-engine wait); the j=0 half
            # of iteration t-1 is normalized on DVE using the then-ready rstd,
            # so the in-order DVE queue never waits on ACT.
            #
            # Remaining score-pairs (blocks 1-3) are fed from a global queue,
            # 2 per tile iteration AFTER that tile's y-work, so the in-order
            # PE queue never parks y matmuls behind exp-paced score matmuls.
            with tc.tile_pool(name="psY", bufs=3, space="PSUM") as psY:
                if taps:
                    nc.gpsimd.dma_start(
                        out=tap_handles["T_EXP0"][:], in_=exp_tiles[0][:]
                    )
                emit_av(0)
                emit_next_pairs(SCHED["prefill"])  # seam pre-fill
                prev = None  # (psy0, out_sb, mv, rstd, t)
                for b in range(NB):
                    if taps and b == NB - 1:
                        nc.gpsimd.dma_start(out=tap_handles["T_AVT"][:], in_=avT8[:])
                    for ti in range(4):
                        t = b * 4 + ti
                        if ti == 0 and b >= SCHED["diag_pf_from"]:
                            emit_block_diags(b)
                        out_sb = outp.tile([128, D], bf16, tag="o")
                        psy = [None, None]
                        stats = work.tile([128, 2, 6], f32, tag="stats")
                        # block 3: the score-psum pool is free; use its 2-bank
                        # tiles for y so stats/norm run 1024-wide and the LN
                        # pipeline gets extra depth
                        psyt = None
                        if (b == NB - 1 and ti % 2 == 0) or (
                            t == 11 and SCHED["t11_psyt"]
                        ):
                            psyt = psS.tile(
                                [128, 2, 512], f32, tag="pss", name=f"psy2_{t}"
                            )
                            psy[0] = psyt[:, 0, :]
                            psy[1] = psyt[:, 1, :]
                        else:
                            psy[0] = psY.tile([128, 512], f32, tag="ps", name=f"psy0_{t}")
                            psy[1] = psY.tile([128, 512], f32, tag="ps", name=f"psy1_{t}")
                        if t in diags:
                            diag_t = diags.pop(t)
                        else:
                            # per-row diag(c): transpose this tile's c chunk
                            # into spare psy columns (overwritten by the y
                            # matmuls right after diag is built)
                            nc.tensor.transpose(
                                out=psy[0][:, 0:32],
                                in_=crowb[:, t * 128 : (t + 1) * 128],
                                identity=ident[0:32, 0:32],
                            )
                            diag_t = work.tile([128, 128], bf16, tag="diag")
                            nc.vector.tensor_scalar(
                                out=diag_t, in0=identb, scalar1=psy[0][:, 0:1],
                                scalar2=None, op0=mybir.AluOpType.mult,
                            )
                        for j in range(2):
                            psy_j = psy[j]
                            nc.tensor.matmul(
                                psy_j,
                                avT8[:, :, t * 128 : (t + 1) * 128],
                                wob8[:, :, j * 512 : (j + 1) * 512],
                                start=True,
                                stop=False,
                                perf_mode=DR,
                            )
                            nc.tensor.matmul(
                                psy_j,
                                diag_t,
                                xb_sb[:, t, j * 512 : (j + 1) * 512],
                                start=False,
                                stop=True,
                            )
                            nc.vector.bn_stats(out=stats[:, j, :], in_=psy_j)
                        n_emit = 3 if t < SCHED["early3"] else 2
                        if SCHED["emit_pos"] == "mid":
                            emit_next_pairs(n_emit)
                        mv = work.tile([128, 2], f32, tag="mv")
                        nc.vector.bn_aggr(out=mv, in_=stats)
                        # ACT-local chain (no DVE hop): mneg, then
                        # rstd = (var'+epsC)^-0.5 = exp(-0.5*ln(var'+epsC))
                        mneg = work.tile([128, 1], f32, tag="mneg")
                        nc.scalar.mul(mneg, mv[:, 0:1], -1.0)
                        lnv = work.tile([128, 1], f32, tag="lnv")
                        nc.scalar.activation(
                            out=lnv, in_=mv[:, 1:2], func=AF.Ln,
                            bias=epsS_t, scale=1.0,
                        )
                        rstd = work.tile([128, 1], f32, tag="rstd")
                        nc.scalar.activation(
                            out=rstd, in_=lnv, func=AF.Exp, scale=-0.5,
                        )
                        # nm = -mu*rstd, on ACT so the chain stays ACT-local
                        nm = work.tile([128, 1], f32, tag="nm")
                        nc.scalar.activation(
                            out=nm, in_=mneg, func=AF.Copy, scale=rstd,
                        )
                        if psyt is None:
                            nc.scalar.activation(
                                out=out_sb[:, 512:1024], in_=psy[1],
                                func=AF.Identity, bias=nm, scale=rstd,
                            )
                        if prev is not None:
                            p_psy0, p_out, p_mv, p_rstd, p_t = prev
                            nc.vector.tensor_scalar(
                                out=p_out[:, 0:512], in0=p_psy0,
                                scalar1=p_mv[:, 0:1], scalar2=p_rstd,
                                op0=mybir.AluOpType.subtract,
                                op1=mybir.AluOpType.mult,
                            )
                            nc.sync.dma_start(
                                out=out_view[:, p_t, :], in_=p_out
                            )
                            prev = None
                        if psyt is not None:
                            # block-3 + t=11 tiles: exps are done, ACT has
                            # slack and DVE (stats-bound) is critical ->
                            # normalize 1024-wide in one ACT op
                            nc.scalar.activation(
                                out=out_sb[:].rearrange(
                                    "p (j d) -> p j d", j=2
                                ),
                                in_=psyt,
                                func=AF.Identity, bias=nm, scale=rstd,
                            )
                            nc.sync.dma_start(
                                out=out_view[:, t, :], in_=out_sb
                            )
                        elif t >= 12:
                            # non-psyt block-3 tiles: j1 was normalized on ACT
                            # above (stored immediately as its own half so the
                            # DMA overlaps DVE's j0 norm); j0 on DVE here
                            nc.sync.dma_start(
                                out=out_view[:, t, 512:1024],
                                in_=out_sb[:, 512:1024],
                            )
                            nc.vector.tensor_scalar(
                                out=out_sb[:, 0:512], in0=psy[0],
                                scalar1=mv[:, 0:1], scalar2=rstd,
                                op0=mybir.AluOpType.subtract,
                                op1=mybir.AluOpType.mult,
                            )
                            nc.sync.dma_start(
                                out=out_view[:, t, 0:512], in_=out_sb[:, 0:512]
                            )
                        elif t >= SCHED["t_act"]:
                            # late non-psyt tiles: exp stream has drained, so
                            # ACT takes the j0 half too (DVE keeps only stats)
                            nc.scalar.activation(
                                out=out_sb[:, 0:512], in_=psy[0],
                                func=AF.Identity, bias=nm, scale=rstd,
                            )
                            nc.sync.dma_start(
                                out=out_view[:, t, :], in_=out_sb
                            )
                        else:
                            prev = (psy[0], out_sb, mv, rstd, t)
                        if ti == (SCHED["av3_ti"] if b == 2 else 2) and b + 1 < NB:
                            emit_av(b + 1)
                        if SCHED["emit_pos"] == "end":
                            emit_next_pairs(n_emit)
                # tail: finish any pending deferred tile
                if prev is not None:
                    p_psy0, p_out, p_mv, p_rstd, p_t = prev
                    nc.vector.tensor_scalar(
                        out=p_out[:, 0:512], in0=p_psy0,
                        scalar1=p_mv[:, 0:1], scalar2=p_rstd,
                        op0=mybir.AluOpType.subtract,
                        op1=mybir.AluOpType.mult,
                    )
                    nc.sync.dma_start(out=out_view[:, p_t, :], in_=p_out)

    nc.compile()
    return nc


def _get_compiled():
    if "nc" not in _COMPILED:
        _COMPILED["nc"] = _build_bass()
    return _COMPILED["nc"]


def _host_inputs(X, Wq, bq, Wk, bk, Wv, bv, Wo, bo):
    import ml_dtypes

    f8 = ml_dtypes.float8_e4m3
    bf = ml_dtypes.bfloat16
    f32 = np.float32

    # [D, 192] = [q|k|v] weights -> [128, 8, 192] with d = c*128 + p
    wqkv = np.concatenate([Wq, Wk, Wv], axis=1).astype(f32)
    wqkv8 = np.ascontiguousarray(
        wqkv.reshape(NC_, 128, 192).transpose(1, 0, 2)
    ).astype(f8)
    bqk = np.concatenate([bq, bk]).astype(f32)
    wob8 = np.zeros((33, 2, D), dtype=f8)
    wo8 = (Wo.astype(f32) * WOS).astype(f8)
    wob8[:32, 0, :] = wo8[0:32]
    wob8[:32, 1, :] = wo8[32:64]
    # row 32 pairs with the c-row (sums/32) in avT8: contributes c*bo
    wob8[32, 0, :] = bo.astype(f32).astype(f8)

    common = {
        "WQKV8": wqkv8,
        "BQK": bqk,
        "BV": bv.astype(f32).astype(bf),
        "WOB8": wob8,
    }
    per_core = []
    for i in range(X.shape[0]):
        Xi = np.ascontiguousarray(X[i], dtype=f32)
        per_core.append(
            {
                "XB": Xi.astype(bf),
                "XT8": np.ascontiguousarray(Xi.T).astype(f8),
                **common,
            }
        )
    return per_core


def kernel(X, Wq, bq, Wk, bk, Wv, bv, Wo, bo, gamma, beta):
    from concourse.bass_utils import run_bass_kernel_spmd

    X = np.asarray(X, dtype=np.float32)
    gamma_np = np.asarray(gamma, dtype=np.float32)
    beta_np = np.asarray(beta, dtype=np.float32)

    nc = _get_compiled()
    in_maps = _host_inputs(
        X,
        np.asarray(Wq), np.asarray(bq), np.asarray(Wk), np.asarray(bk),
        np.asarray(Wv), np.asarray(bv), np.asarray(Wo), np.asarray(bo),
    )
    res = run_bass_kernel_spmd(nc, in_maps, core_ids=list(range(B)))
    out = np.stack(
        [np.asarray(res.results[i]["OUT"]).astype(np.float32) for i in range(B)],
        axis=0,
    )
    if not (np.all(gamma_np == 1.0) and np.all(beta_np == 0.0)):
        out = out * gamma_np + beta_np
    return out.astype(np.float32)



# revision 74
# speedup vs baseline: 1.0060x; 1.0008x over previous
"""Trainium2 Bass kernel v3: batched single-head attention + residual + layernorm.

Per batch element b (one NeuronCore each, data-parallel over B=8):
    q = X@Wq+bq; k = X@Wk+bk; v = X@Wv+bv          [S=2048, K=64]
    attn = softmax(q @ k.T / 8, axis=-1)            [S, S]
    y = X + (attn @ v) @ Wo + bo                    [S, D=1024]
    out = layernorm(y) * gamma + beta

v3 design (v2 + timeline-driven scheduling/work changes):
  - Host passes X twice: X.T fp8e4m3 (projections) and X bf16 (residual);
    q/k/v weights packed into ONE [128,8,192] tensor whose rows are 1536B
    contiguous (single cheap DMA). Load plan: weights + two 1MB xt8 halves
    on SP/HWDGE; biases, wob8 and xb (8 chunks) on the Pool/SWDGE queue.
  - PE p-state warmup: ~88 dependency-free identity matmuls through the
    xt8-load window ramp the Tensor engine to full clock before the
    projections (cost model halves matmul speed for the first 3us of a
    busy streak).
  - Projections fp8 DoubleRow; block-0 k copy runs on ACT in parallel with
    DVE's q copy; scores bf16; exp on ACT 1024-wide into fp8 expT.
  - attn@v: fp8 DoubleRow with an extra v column of 1/64 accumulating
    sums/64 into psu row 64.
  - NO softmax division anywhere: layernorm is invariant to a positive
    per-row scale, so psy holds c*y with c = sums/32. av8 = uav/256 (fp8),
    the c-row joins avT8 row 32 (pairs with wob8 row 32 = bo), and X enters
    via diag(c) @ X - diag built per tile from a PE transpose of the c-row
    chunk into spare PSUM columns + a DVE identity-scale. rstd comes
    directly from the scaled stats: rstd' = (var' + 1e-3)^-0.5 via Ln/Exp.
  - LN engine split tuned from the simulated timeline: early tiles defer
    the j0 half to DVE one iteration later (ACT is exp-saturated); tiles
    >= 5 normalize fully on ACT; block-3 tiles alternate 2-bank PSUM tiles
    (1024-wide single-op ACT norms) with 1-bank pairs so two tiles stay in
    flight; block-3 diag tiles are prebuilt at iteration-12.
  - Score-pair emission: 8 pairs during projections (tgt 0), 6-pair seam
    prefill, then 2 per y-iteration, keeping ACT's exp stream saturated
    without parking PE's in-order queue on score PSUM slots.

gamma/beta are ones/zeros for this problem; applied on host if non-trivial.
"""

import numpy as np

B = 8
S = 2048
D = 1024
K = 64
EPS = 1e-5

NT = S // 128   # 16 s-tiles
NC_ = D // 128  # 8 d-chunks
NB = S // 512   # 4 query blocks

YS = 512.0      # psy = 512*y
AVS = 64.0      # av8 = 64*av ; v sums col = 1/64
WOS = 8.0       # wob8 = 8*Wo rows

_COMPILED = {}

# scheduling knobs (read at build time)
SCHED = {
    "emit_pos": "end",   # "mid": after stats j-loop; "end": end of iteration
    "early3": 0,          # iterations with 3 emissions (rest get 2)
    "av3_ti": 2,          # ti at which emit_av(3) fires (b==2)
    "prefill": 7,
    "outp_bufs": 4,
    "work_bufs": 4,
    "fillers": 88,
    "t_act": 5,
    "t11_psyt": True,
    "diag_pf_from": 3,
}


def _build_bass(act_norm_tiles=8, taps=False):
    import concourse.bacc as bacc
    import concourse.tile as tile
    from concourse import mybir
    from concourse.masks import make_identity

    f32 = mybir.dt.float32
    f32r = mybir.dt.float32r
    bf16 = mybir.dt.bfloat16
    f8 = mybir.dt.float8e4
    AF = mybir.ActivationFunctionType
    DR = mybir.MatmulPerfMode.DoubleRow

    nc = bacc.Bacc("TRN2", target_bir_lowering=False, debug=False)

    xb_dram = nc.dram_tensor("XB", [S, D], bf16, kind="ExternalInput")
    xt8_dram = nc.dram_tensor("XT8", [D, S], f8, kind="ExternalInput")
    # all projection weights packed host-side into one [128, 8, 192] tensor:
    # cols 0:64 q, 64:128 k, 128:192 v per d-chunk -> ONE 546ns DMA with
    # 1536B-contiguous rows instead of 4 small strided loads.
    wqkv8_dram = nc.dram_tensor("WQKV8", [128, NC_, 192], f8, kind="ExternalInput")
    bqk_dram = nc.dram_tensor("BQK", [128], f32, kind="ExternalInput")
    bv_dram = nc.dram_tensor("BV", [K], bf16, kind="ExternalInput")
    wob8_dram = nc.dram_tensor("WOB8", [33, 2, D], f8, kind="ExternalInput")
    out_dram = nc.dram_tensor("OUT", [S, D], bf16, kind="ExternalOutput")
    tap_handles = {}
    if taps:
        for name, shape, dt_ in [
            ("T_QK", [K, 2, S], mybir.dt.bfloat16),
            ("T_V", [128, NT, K + 1], mybir.dt.float8e4),
            ("T_EXP0", [128, NT, 512], mybir.dt.float8e4),
            ("T_AVT", [33, 2, S], mybir.dt.float8e4),
            ("T_RECB", [K, 512], mybir.dt.float32),
        ]:
            tap_handles[name] = nc.dram_tensor(name, shape, dt_, kind="ExternalOutput")

    with tile.TileContext(nc) as tc:
        with (
            tc.tile_pool(name="consts", bufs=1) as consts,
            tc.tile_pool(name="bigx", bufs=1) as bigx,
            tc.tile_pool(name="proj", bufs=1) as proj,
            tc.tile_pool(name="vtp", bufs=2) as vtp,
            tc.tile_pool(name="avn", bufs=SCHED.get("avn_bufs", 2)) as avn,
            tc.tile_pool(name="outp", bufs=SCHED["outp_bufs"]) as outp,
            tc.tile_pool(name="work", bufs=SCHED["work_bufs"]) as work,
            tc.tile_pool(name="expp", bufs=2) as expp,
            tc.tile_pool(name="psS", bufs=2, space="PSUM") as psS,
            tc.tile_pool(name="psU", bufs=1, space="PSUM") as psU,
        ):
            # Pre-place the act table that serves Exp+Ln+Identity+Copy so the
            # compiler's table-load pass doesn't flip-flop between the
            # exp-only and ln-only tables (1283ns per reload).
            nc.scalar.add_instruction(
                mybir.InstLoadActFuncSet(
                    name=nc.get_next_instruction_name(),
                    ins=[], outs=[], act_func_set_id=6,
                )
            )
            ident = consts.tile([128, 128], f32)
            make_identity(nc, ident)
            identb = consts.tile([128, 128], bf16)
            nc.gpsimd.tensor_copy(out=identb, in_=ident)
            # LN is invariant to a positive per-row scale, so psy holds
            # c*y with c = sums/32 (the softmax denominator, folded into the
            # diagonal of the residual matmul): rstd' = (var' + epsC)^-0.5
            # normalizes exactly. epsC approximates c^2*EPS; the mismatch is
            # <0.02% of var'.
            epsS_t = consts.tile([128, 1], f32)
            nc.vector.memset(epsS_t, 1e-3)
            ones_f = consts.tile([128, 512], f32)
            nc.gpsimd.memset(ones_f, 1.0)
            ones512r = consts.tile([1, 512], f32r)
            with nc.allow_low_precision(reason="ones constant to f32r"):
                nc.gpsimd.tensor_copy(out=ones512r, in_=ones_f[0:1, :])
            ones_row128 = consts.tile([1, 128], bf16)
            nc.gpsimd.memset(ones_row128, 1.0)

            # Load plan: SP/HWDGE queue carries the packed weights then xt8 in
            # two 1MB transfers (minimal desc-gen serialization -> first q
            # matmul ~4us earlier than 8 chunked loads). Pool/SWDGE carries
            # the small biases + wob8 + xb (desc-gen on the otherwise-idle
            # Pool engine, transfers interleave behind xt8 on the DMA device).
            xt8_sb = bigx.tile([128, NC_, S], f8)
            xt8_view = xt8_dram[:].rearrange("(c p) s -> p c s", p=128)
            xb_sb = bigx.tile([128, NT, D], bf16)
            xb_view = xb_dram[:].rearrange("(t p) d -> p t d", p=128)
            wqkv8 = consts.tile([128, NC_, 192], f8)
            bqk_row = consts.tile([1, 2, K], f32r)  # [q|k] bias as matmul lhsT
            bv_row8 = consts.tile([1, K], bf16)
            wob8 = consts.tile([33, 2, D], f8)
            # SP/HWDGE side: packed weights then the two xt8 halves
            nc.sync.dma_start(out=wqkv8, in_=wqkv8_dram[:])
            nc.sync.dma_start(out=xt8_sb[:, 0:4, :], in_=xt8_view[:, 0:4, :])
            nc.sync.dma_start(out=xt8_sb[:, 4:8, :], in_=xt8_view[:, 4:8, :])
            # Pool/SWDGE side: biases, wob8, then xb in 8 2-tile chunks
            # (small enough to not hog the DMA device at the avT8 seams)
            nc.gpsimd.dma_start(
                out=bqk_row,
                in_=bqk_dram[:].rearrange("(a j k) -> a j k", a=1, j=2).bitcast(f32r),
            )
            nc.gpsimd.dma_start(
                out=bv_row8, in_=bv_dram[:].rearrange("(a k) -> a k", a=1)
            )
            nc.gpsimd.dma_start(out=wob8, in_=wob8_dram[:])
            for h in range(8):
                nc.gpsimd.dma_start(
                    out=xb_sb[:, 2 * h : 2 * h + 2, :],
                    in_=xb_view[:, 2 * h : 2 * h + 2, :],
                )

            # PE p-state warmup: the cost model halves matmul throughput for
            # the first 3us of any busy streak (and quarters it at streak
            # start). A chain of dependency-free identity matmuls through the
            # xt8-load window ramps PE to full clock before the projections,
            # and keeps the streak alive until xt8's second half lands.
            warm = psS.tile([128, 2, 512], f32, tag="pss", name="warm")
            for _ in range(SCHED["fillers"]):
                nc.tensor.matmul(
                    warm[:, 0, 0:128], identb, identb,
                    start=True, stop=True,
                )

            qk2_sb = proj.tile([K, 2, S], bf16)  # [:,0,:] q, [:,1,:] k
            v_sb = proj.tile([128, NT, 80], f8)  # cols 0:64 v, col 64 = 1/64
            nc.gpsimd.memset(v_sb[:, :, K : K + 1], 1.0 / AVS)
            # avT8 rows 0:32 = uav/256 halves; row 32 ch0 = c-row (sums/32,
            # written per block from crowb), ch1 = 0 so its wob8 row is inert
            avT8 = proj.tile([33, 2, S], f8)
            nc.gpsimd.memset(avT8[32:33, 1, :], 0.0)

            exp_tiles = {}

            def emit_scores(tgt, pair_list):
                if tgt not in exp_tiles:
                    et = expp.tile([128, NT, 512], f8, tag="expT", name=f"expT{tgt}")
                    exp_tiles[tgt] = et
                et = exp_tiles[tgt]
                sqt = slice(tgt * 512, (tgt + 1) * 512)
                for p in pair_list:
                    pss = psS.tile([128, 2, 512], f32, tag="pss", name=f"pss{tgt}_{p}")
                    for j in range(2):
                        sk = 2 * p + j
                        nc.tensor.matmul(
                            pss[:, j, :],
                            qk2_sb[:, 1, sk * 128 : (sk + 1) * 128],
                            qk2_sb[:, 0, sqt],
                            start=True,
                            stop=True,
                        )
                    nc.scalar.activation(
                        out=et[:, 2 * p : 2 * p + 2, :], in_=pss[:],
                        func=AF.Exp, scale=0.125,
                    )

            pair_queue = [(tgt, p) for tgt in range(1, NB) for p in range(NT // 2)]

            def emit_next_pairs(n):
                for _ in range(n):
                    if pair_queue:
                        tgt, p = pair_queue.pop(0)
                        emit_scores(tgt, [p])

            # ---- phase 1: projections; block-0 scores piped in, block-1
            # scores emitted after (so expT(0) completes as early as possible
            # and phase 2 can start under the block-1 exp stream) ----
            with tc.tile_pool(name="psP", bufs=3, space="PSUM") as psP:
                for b in range(NB):
                    sq = slice(b * 512, (b + 1) * 512)
                    # q and k as separate 64-col chains, single-bank tiles
                    # (both land on partitions 0:64; DVE copies cannot cross
                    # partitions); biases folded in as ones-row matmuls.
                    # Block 0 interleaves the q/k chains across the two xt8
                    # DMA halves so PE starts as soon as half 1 lands.
                    psqk = [
                        psP.tile([K, 512], f32, tag=tg, bufs=1, name=f"ps{tg}")
                        for tg in ("psq", "psk")
                    ]
                    for j, cp in [(j, cp) for j in range(2) for cp in range(4)]:
                        nc.tensor.matmul(
                            psqk[j],
                            wqkv8[:, 2 * cp : 2 * cp + 2, j * K : (j + 1) * K],
                            xt8_sb[:, 2 * cp : 2 * cp + 2, sq],
                            start=(cp == 0),
                            stop=False,
                            perf_mode=DR,
                        )
                    for j in range(2):
                        nc.tensor.matmul(
                            psqk[j],
                            bqk_row[:, j, :],
                            ones512r,
                            start=False,
                            stop=True,
                        )
                        if b == 0 and j == 1:
                            # block 0 is latency-critical for the first exp:
                            # run the k copy on the still-idle ACT engine in
                            # parallel with DVE's q copy
                            nc.scalar.activation(
                                out=qk2_sb[:, j, sq], in_=psqk[j],
                                func=AF.Identity,
                            )
                        else:
                            nc.vector.tensor_copy(out=qk2_sb[:, j, sq], in_=psqk[j])
                    # scores for block 0 as its k-tiles become available;
                    # emitted before the v matmuls so the exp stream starts
                    # as early as possible; blocks 1-2 also pull 2 pairs from
                    # the global queue (ACT otherwise starves mid-projection)
                    emit_scores(0, range(b * 2, b * 2 + 2))
                    # v in natural [s, j] layout: xt8 chunks stationary,
                    # wv8 chunk moving; bias via a ones-row matmul
                    psv = psP.tile([128, 4, K], f32, tag="psv", bufs=1)
                    for ti in range(4):
                        t = b * 4 + ti
                        for c in range(NC_):
                            nc.tensor.matmul(
                                psv[:, ti, :],
                                xt8_sb[:, c, t * 128 : (t + 1) * 128],
                                wqkv8[:, c, 128:192],
                                start=(c == 0),
                                stop=False,
                            )
                        nc.tensor.matmul(
                            psv[:, ti, :],
                            ones_row128,
                            bv_row8,
                            start=False,
                            stop=True,
                        )
                    nc.vector.tensor_copy(
                        out=v_sb[:, b * 4 : (b + 1) * 4, 0:K], in_=psv
                    )

            # ---- phase 2 ----
            out_view = out_dram[:].rearrange("(t p) d -> p t d", p=128)

            # c-row staging: row 0 carries c = sums/32 per block; rows 1:31
            # stay zero so the per-tile PE transpose of [32,128] chunks reads
            # defined data
            crowb = proj.tile([32, S], f32)
            nc.gpsimd.memset(crowb[:, :], 0.0)

            diags = {}
            psu_by_block = {}

            def emit_block_diags(b):
                # prebuild block b's four diag(c) tiles (transposes into
                # psu_b's spare columns). The per-tile diag otherwise queues
                # behind DVE's stats backlog and gates every y matmul.
                # Called at iteration-(4b) top so the PE park on the c-row
                # cast sits AFTER the previous block's matmuls.
                psu = psu_by_block.pop(b)
                for ti in range(4):
                    t = 4 * b + ti
                    cs = 384 + 32 * ti
                    nc.tensor.transpose(
                        out=psu[:, cs : cs + 32],
                        in_=crowb[:, t * 128 : (t + 1) * 128],
                        identity=ident[0:32, 0:32],
                    )
                    dt_ = work.tile(
                        [128, 128], bf16, tag="diag", name=f"diag{t}"
                    )
                    nc.vector.tensor_scalar(
                        out=dt_, in0=identb, scalar1=psu[:, cs : cs + 1],
                        scalar2=None, op0=mybir.AluOpType.mult,
                    )
                    diags[t] = dt_

            def emit_av(b, splits=1):
                """uav -> av8 = uav/256 cast + c-row = sums/32 for block b,
                then ALL FOUR of the block's diag(c) tiles. Prebuilding the
                diagonals at the seam keeps them out of the per-tile DVE
                queue, whose stats backlog otherwise gates the y matmuls.
                No softmax division: LN's scale-invariance absorbs the
                denominator via the per-row c in the residual diagonal."""
                expT = exp_tiles.pop(b)
                psu = psU.tile([128, 512], f32, tag="psu", name=f"psu{b}")
                for tp in range(NT // 2):
                    nc.tensor.matmul(
                        psu[0 : K + 1, :],
                        v_sb[:, 2 * tp : 2 * tp + 2, 0 : K + 1],
                        expT[:, 2 * tp : 2 * tp + 2, :],
                        start=(tp == 0),
                        stop=(tp == NT // 2 - 1),
                        perf_mode=DR,
                    )
                av8 = avn.tile([K, 512], f8, tag="av8")
                sq = slice(b * 512, (b + 1) * 512)
                with nc.allow_low_precision(reason="uav cast to f8"):
                    nc.vector.tensor_scalar(
                        out=av8, in0=psu[0:K, :], scalar1=1.0 / 256.0,
                        scalar2=None, op0=mybir.AluOpType.mult,
                    )
                nc.vector.tensor_scalar(
                    out=crowb[0:1, sq], in0=psu[K : K + 1, :], scalar1=2.0,
                    scalar2=None, op0=mybir.AluOpType.mult,
                )
                nc.sync.dma_start(out=avT8[0:32, 0, sq], in_=av8[0:32, :])
                nc.sync.dma_start(out=avT8[0:32, 1, sq], in_=av8[32:K, :])
                # c-row into avT8 on Pool (SBUF->SBUF bf16->f8 convert)
                with nc.allow_low_precision(reason="c-row cast to f8"):
                    nc.gpsimd.tensor_copy(
                        out=avT8[32:33, 0, sq], in_=crowb[0:1, sq]
                    )
                psu_by_block[b] = psu
                if taps and b == 0:
                    nc.gpsimd.dma_start(out=tap_handles["T_RECB"][:], in_=crowb[0:8, 0:512])

            if taps:
                nc.gpsimd.dma_start(out=tap_handles["T_QK"][:], in_=qk2_sb[:])
                nc.gpsimd.dma_start(out=tap_handles["T_V"][:], in_=v_sb[:, :, 0 : K + 1])
            # Software-pipelined LN: at iteration t, the j=1 half is normalized
            # on ACT (same engine as rstd, no cross-engine wait); the j=0 half
            # of iteration t-1 is normalized on DVE using the then-ready rstd,
            # so the in-order DVE queue never waits on ACT.
            #
            # Remaining score-pairs (blocks 1-3) are fed from a global queue,
            # 2 per tile iteration AFTER that tile's y-work, so the in-order
            # PE queue never parks y matmuls behind exp-paced score matmuls.
            with tc.tile_pool(name="psY", bufs=3, space="PSUM") as psY:
                if taps:
                    nc.gpsimd.dma_start(
                        out=tap_handles["T_EXP0"][:], in_=exp_tiles[0][:]
                    )
                emit_av(0)
                emit_next_pairs(SCHED["prefill"])  # seam pre-fill
                prev = None  # (psy0, out_sb, mv, rstd, t)
                for b in range(NB):
                    if taps and b == NB - 1:
                        nc.gpsimd.dma_start(out=tap_handles["T_AVT"][:], in_=avT8[:])
                    for ti in range(4):
                        t = b * 4 + ti
                        if ti == 0 and b >= SCHED["diag_pf_from"]:
                            emit_block_diags(b)
                        out_sb = outp.tile([128, D], bf16, tag="o")
                        psy = [None, None]
                        stats = work.tile([128, 2, 6], f32, tag="stats")
                        # block 3: the score-psum pool is free; use its 2-bank
                        # tiles for y so stats/norm run 1024-wide and the LN
                        # pipeline gets extra depth
                        psyt = None
                        if (b == NB - 1 and ti % 2 == 0) or (
                            t == 11 and SCHED["t11_psyt"]
                        ):
                            psyt = psS.tile(
                                [128, 2, 512], f32, tag="pss", name=f"psy2_{t}"
                            )
                            psy[0] = psyt[:, 0, :]
                            psy[1] = psyt[:, 1, :]
                        else:
                            psy[0] = psY.tile([128, 512], f32, tag="ps", name=f"psy0_{t}")
                            psy[1] = psY.tile([128, 512], f32, tag="ps", name=f"psy1_{t}")
                        if t in diags:
                            diag_t = diags.pop(t)
                        else:
                            # per-row diag(c): transpose this tile's c chunk
                            # into spare psy columns (overwritten by the y
                            # matmuls right after diag is built)
                            nc.tensor.transpose(
                                out=psy[0][:, 0:32],
                                in_=crowb[:, t * 128 : (t + 1) * 128],
                                identity=ident[0:32, 0:32],
                            )
                            diag_t = work.tile([128, 128], bf16, tag="diag")
                            nc.vector.tensor_scalar(
                                out=diag_t, in0=identb, scalar1=psy[0][:, 0:1],
                                scalar2=None, op0=mybir.AluOpType.mult,
                            )
                        for j in range(2):
                            psy_j = psy[j]
                            nc.tensor.matmul(
                                psy_j,
                                avT8[:, :, t * 128 : (t + 1) * 128],
                                wob8[:, :, j * 512 : (j + 1) * 512],
                                start=True,
                                stop=False,
                                perf_mode=DR,
                            )
                            nc.tensor.matmul(
                                psy_j,
                                diag_t,
                                xb_sb[:, t, j * 512 : (j + 1) * 512],
                                start=False,
                                stop=True,
                            )
                            nc.vector.bn_stats(out=stats[:, j, :], in_=psy_j)
                        n_emit = 3 if t < SCHED["early3"] else 2
                        if SCHED["emit_pos"] == "mid":
                            emit_next_pairs(n_emit)
                        mv = work.tile([128, 2], f32, tag="mv")
                        nc.vector.bn_aggr(out=mv, in_=stats)
                        # ACT-local chain (no DVE hop): mneg, then
                        # rstd = (var'+epsC)^-0.5 = exp(-0.5*ln(var'+epsC))
                        mneg = work.tile([128, 1], f32, tag="mneg")
                        nc.scalar.mul(mneg, mv[:, 0:1], -1.0)
                        lnv = work.tile([128, 1], f32, tag="lnv")
                        nc.scalar.activation(
                            out=lnv, in_=mv[:, 1:2], func=AF.Ln,
                            bias=epsS_t, scale=1.0,
                        )
                        rstd = work.tile([128, 1], f32, tag="rstd")
                        nc.scalar.activation(
                            out=rstd, in_=lnv, func=AF.Exp, scale=-0.5,
                        )
                        # nm = -mu*rstd, on ACT so the chain stays ACT-local
                        nm = work.tile([128, 1], f32, tag="nm")
                        nc.scalar.activation(
                            out=nm, in_=mneg, func=AF.Copy, scale=rstd,
                        )
                        if psyt is None:
                            nc.scalar.activation(
                                out=out_sb[:, 512:1024], in_=psy[1],
                                func=AF.Identity, bias=nm, scale=rstd,
                            )
                        if prev is not None:
                            p_psy0, p_out, p_mv, p_rstd, p_t = prev
                            nc.vector.tensor_scalar(
                                out=p_out[:, 0:512], in0=p_psy0,
                                scalar1=p_mv[:, 0:1], scalar2=p_rstd,
                                op0=mybir.AluOpType.subtract,
                                op1=mybir.AluOpType.mult,
                            )
                            nc.sync.dma_start(
                                out=out_view[:, p_t, :], in_=p_out
                            )
                            prev = None
                        if psyt is not None:
                            # block-3 + t=11 tiles: exps are done, ACT has
                            # slack and DVE (stats-bound) is critical ->
                            # normalize 1024-wide in one ACT op
                            nc.scalar.activation(
                                out=out_sb[:].rearrange(
                                    "p (j d) -> p j d", j=2
                                ),
                                in_=psyt,
                                func=AF.Identity, bias=nm, scale=rstd,
                            )
                            nc.sync.dma_start(
                                out=out_view[:, t, :], in_=out_sb
                            )
                        elif t >= 12:
                            # non-psyt block-3 tiles: j1 was normalized on ACT
                            # above (stored immediately as its own half so the
                            # DMA overlaps DVE's j0 norm); j0 on DVE here
                            nc.sync.dma_start(
                                out=out_view[:, t, 512:1024],
                                in_=out_sb[:, 512:1024],
                            )
                            nc.vector.tensor_scalar(
                                out=out_sb[:, 0:512], in0=psy[0],
                                scalar1=mv[:, 0:1], scalar2=rstd,
                                op0=mybir.AluOpType.subtract,
                                op1=mybir.AluOpType.mult,
                            )
                            # j0 store from the ACT queue (empty at the
                            # tail), skipping SP's head-of-line descriptor
                            # backlog
                            nc.scalar.dma_start(
                                out=out_view[:, t, 0:512], in_=out_sb[:, 0:512]
                            )
                        elif t >= SCHED["t_act"]:
                            # late non-psyt tiles: exp stream has drained, so
                            # ACT takes the j0 half too (DVE keeps only stats)
                            nc.scalar.activation(
                                out=out_sb[:, 0:512], in_=psy[0],
                                func=AF.Identity, bias=nm, scale=rstd,
                            )
                            nc.sync.dma_start(
                                out=out_view[:, t, :], in_=out_sb
                            )
                        else:
                            prev = (psy[0], out_sb, mv, rstd, t)
                        if ti == (SCHED["av3_ti"] if b == 2 else 2) and b + 1 < NB:
                            emit_av(b + 1)
                        if SCHED["emit_pos"] == "end":
                            emit_next_pairs(n_emit)
                # tail: finish any pending deferred tile
                if prev is not None:
                    p_psy0, p_out, p_mv, p_rstd, p_t = prev
                    nc.vector.tensor_scalar(
                        out=p_out[:, 0:512], in0=p_psy0,
                        scalar1=p_mv[:, 0:1], scalar2=p_rstd,
                        op0=mybir.AluOpType.subtract,
                        op1=mybir.AluOpType.mult,
                    )
                    nc.sync.dma_start(out=out_view[:, p_t, :], in_=p_out)

    nc.compile()
    return nc


def _get_compiled():
    if "nc" not in _COMPILED:
        _COMPILED["nc"] = _build_bass()
    return _COMPILED["nc"]


def _host_inputs(X, Wq, bq, Wk, bk, Wv, bv, Wo, bo):
    import ml_dtypes

    f8 = ml_dtypes.float8_e4m3
    bf = ml_dtypes.bfloat16
    f32 = np.float32

    # [D, 192] = [q|k|v] weights -> [128, 8, 192] with d = c*128 + p
    wqkv = np.concatenate([Wq, Wk, Wv], axis=1).astype(f32)
    wqkv8 = np.ascontiguousarray(
        wqkv.reshape(NC_, 128, 192).transpose(1, 0, 2)
    ).astype(f8)
    bqk = np.concatenate([bq, bk]).astype(f32)
    wob8 = np.zeros((33, 2, D), dtype=f8)
    wo8 = (Wo.astype(f32) * WOS).astype(f8)
    wob8[:32, 0, :] = wo8[0:32]
    wob8[:32, 1, :] = wo8[32:64]
    # row 32 pairs with the c-row (sums/32) in avT8: contributes c*bo
    wob8[32, 0, :] = bo.astype(f32).astype(f8)

    common = {
        "WQKV8": wqkv8,
        "BQK": bqk,
        "BV": bv.astype(f32).astype(bf),
        "WOB8": wob8,
    }
    per_core = []
    for i in range(X.shape[0]):
        Xi = np.ascontiguousarray(X[i], dtype=f32)
        per_core.append(
            {
                "XB": Xi.astype(bf),
                "XT8": np.ascontiguousarray(Xi.T).astype(f8),
                **common,
            }
        )
    return per_core


def kernel(X, Wq, bq, Wk, bk, Wv, bv, Wo, bo, gamma, beta):
    from concourse.bass_utils import run_bass_kernel_spmd

    X = np.asarray(X, dtype=np.float32)
    gamma_np = np.asarray(gamma, dtype=np.float32)
    beta_np = np.asarray(beta, dtype=np.float32)

    nc = _get_compiled()
    in_maps = _host_inputs(
        X,
        np.asarray(Wq), np.asarray(bq), np.asarray(Wk), np.asarray(bk),
        np.asarray(Wv), np.asarray(bv), np.asarray(Wo), np.asarray(bo),
    )
    res = run_bass_kernel_spmd(nc, in_maps, core_ids=list(range(B)))
    out = np.stack(
        [np.asarray(res.results[i]["OUT"]).astype(np.float32) for i in range(B)],
        axis=0,
    )
    if not (np.all(gamma_np == 1.0) and np.all(beta_np == 0.0)):
        out = out * gamma_np + beta_np
    return out.astype(np.float32)



# revision 75
# speedup vs baseline: 1.0208x; 1.0148x over previous
"""Trainium2 Bass kernel v3: batched single-head attention + residual + layernorm.

Per batch element b (one NeuronCore each, data-parallel over B=8):
    q = X@Wq+bq; k = X@Wk+bk; v = X@Wv+bv          [S=2048, K=64]
    attn = softmax(q @ k.T / 8, axis=-1)            [S, S]
    y = X + (attn @ v) @ Wo + bo                    [S, D=1024]
    out = layernorm(y) * gamma + beta

v3 design (v2 + timeline-driven scheduling/work changes):
  - Host passes X twice: X.T fp8e4m3 (projections) and X bf16 (residual);
    q/k/v weights packed into ONE [128,8,192] tensor whose rows are 1536B
    contiguous (single cheap DMA). Load plan: weights + two 1MB xt8 halves
    on SP/HWDGE; biases, wob8 and xb (8 chunks) on the Pool/SWDGE queue.
  - PE p-state warmup: ~88 dependency-free identity matmuls through the
    xt8-load window ramp the Tensor engine to full clock before the
    projections (cost model halves matmul speed for the first 3us of a
    busy streak).
  - Projections fp8 DoubleRow; block-0 k copy runs on ACT in parallel with
    DVE's q copy; scores bf16; exp on ACT 1024-wide into fp8 expT.
  - attn@v: fp8 DoubleRow with an extra v column of 1/64 accumulating
    sums/64 into psu row 64.
  - NO softmax division anywhere: layernorm is invariant to a positive
    per-row scale, so psy holds c*y with c = sums/32. av8 = uav/256 (fp8),
    the c-row joins avT8 row 32 (pairs with wob8 row 32 = bo), and X enters
    via diag(c) @ X - diag built per tile from a PE transpose of the c-row
    chunk into spare PSUM columns + a DVE identity-scale. rstd comes
    directly from the scaled stats: rstd' = (var' + 1e-3)^-0.5 via Ln/Exp.
  - LN engine split tuned from the simulated timeline: early tiles defer
    the j0 half to DVE one iteration later (ACT is exp-saturated); tiles
    >= 5 normalize fully on ACT; block-3 tiles alternate 2-bank PSUM tiles
    (1024-wide single-op ACT norms) with 1-bank pairs so two tiles stay in
    flight; block-3 diag tiles are prebuilt at iteration-12.
  - Score-pair emission: 8 pairs during projections (tgt 0), 6-pair seam
    prefill, then 2 per y-iteration, keeping ACT's exp stream saturated
    without parking PE's in-order queue on score PSUM slots.

gamma/beta are ones/zeros for this problem; applied on host if non-trivial.
"""

import numpy as np

B = 8
S = 2048
D = 1024
K = 64
EPS = 1e-5

NT = S // 128   # 16 s-tiles
NC_ = D // 128  # 8 d-chunks
NB = S // 512   # 4 query blocks

YS = 512.0      # psy = 512*y
AVS = 64.0      # av8 = 64*av ; v sums col = 1/64
WOS = 8.0       # wob8 = 8*Wo rows

_COMPILED = {}

# scheduling knobs (read at build time)
SCHED = {
    "emit_pos": "end",   # "mid": after stats j-loop; "end": end of iteration
    "early3": 0,          # iterations with 3 emissions (rest get 2)
    "av3_ti": 2,          # ti at which emit_av(3) fires (b==2)
    "prefill": 7,
    "outp_bufs": 4,
    "work_bufs": 4,
    "fillers": 88,
    "t_act": 5,
    "t11_psyt": True,
    "diag_pf_from": 3,
}


def _build_bass(act_norm_tiles=8, taps=False):
    import concourse.bacc as bacc
    import concourse.tile as tile
    from concourse import mybir
    from concourse.masks import make_identity

    f32 = mybir.dt.float32
    f32r = mybir.dt.float32r
    bf16 = mybir.dt.bfloat16
    f8 = mybir.dt.float8e4
    AF = mybir.ActivationFunctionType
    DR = mybir.MatmulPerfMode.DoubleRow

    nc = bacc.Bacc("TRN2", target_bir_lowering=False, debug=False)

    xb_dram = nc.dram_tensor("XB", [S, D], bf16, kind="ExternalInput")
    xt8_dram = nc.dram_tensor("XT8", [D, S], f8, kind="ExternalInput")
    # all projection weights packed host-side into one [128, 8, 192] tensor:
    # cols 0:64 q, 64:128 k, 128:192 v per d-chunk -> ONE 546ns DMA with
    # 1536B-contiguous rows instead of 4 small strided loads.
    wqkv8_dram = nc.dram_tensor("WQKV8", [128, NC_, 192], f8, kind="ExternalInput")
    bqk_dram = nc.dram_tensor("BQK", [128], f32, kind="ExternalInput")
    bv_dram = nc.dram_tensor("BV", [K], bf16, kind="ExternalInput")
    wob8_dram = nc.dram_tensor("WOB8", [33, 2, D], f8, kind="ExternalInput")
    out_dram = nc.dram_tensor("OUT", [S, D], bf16, kind="ExternalOutput")
    tap_handles = {}
    if taps:
        for name, shape, dt_ in [
            ("T_QK", [K, 2, S], mybir.dt.bfloat16),
            ("T_V", [128, NT, K + 1], mybir.dt.float8e4),
            ("T_EXP0", [128, NT, 512], mybir.dt.float8e4),
            ("T_AVT", [33, 2, S], mybir.dt.float8e4),
            ("T_RECB", [K, 512], mybir.dt.float32),
        ]:
            tap_handles[name] = nc.dram_tensor(name, shape, dt_, kind="ExternalOutput")

    with tile.TileContext(nc) as tc:
        with (
            tc.tile_pool(name="consts", bufs=1) as consts,
            tc.tile_pool(name="bigx", bufs=1) as bigx,
            tc.tile_pool(name="proj", bufs=1) as proj,
            tc.tile_pool(name="vtp", bufs=2) as vtp,
            tc.tile_pool(name="avn", bufs=SCHED.get("avn_bufs", 2)) as avn,
            tc.tile_pool(name="outp", bufs=SCHED["outp_bufs"]) as outp,
            tc.tile_pool(name="work", bufs=SCHED["work_bufs"]) as work,
            tc.tile_pool(name="expp", bufs=2) as expp,
            tc.tile_pool(name="psS", bufs=2, space="PSUM") as psS,
            tc.tile_pool(name="psU", bufs=1, space="PSUM") as psU,
        ):
            # Pre-place the act table that serves Exp+Ln+Identity+Copy so the
            # compiler's table-load pass doesn't flip-flop between the
            # exp-only and ln-only tables (1283ns per reload).
            nc.scalar.add_instruction(
                mybir.InstLoadActFuncSet(
                    name=nc.get_next_instruction_name(),
                    ins=[], outs=[], act_func_set_id=6,
                )
            )
            ident = consts.tile([128, 128], f32)
            make_identity(nc, ident)
            identb = consts.tile([128, 128], bf16)
            nc.gpsimd.tensor_copy(out=identb, in_=ident)
            # LN is invariant to a positive per-row scale, so psy holds
            # c*y with c = sums/32 (the softmax denominator, folded into the
            # diagonal of the residual matmul): rstd' = (var' + epsC)^-0.5
            # normalizes exactly. epsC approximates c^2*EPS; the mismatch is
            # <0.02% of var'.
            epsS_t = consts.tile([128, 1], f32)
            nc.vector.memset(epsS_t, 1e-3)
            ones_f = consts.tile([128, 512], f32)
            nc.gpsimd.memset(ones_f, 1.0)
            ones512r = consts.tile([1, 512], f32r)
            with nc.allow_low_precision(reason="ones constant to f32r"):
                nc.gpsimd.tensor_copy(out=ones512r, in_=ones_f[0:1, :])
            ones_row128 = consts.tile([1, 128], bf16)
            nc.gpsimd.memset(ones_row128, 1.0)

            # Load plan: SP/HWDGE queue carries the packed weights then xt8 in
            # two 1MB transfers (minimal desc-gen serialization -> first q
            # matmul ~4us earlier than 8 chunked loads). Pool/SWDGE carries
            # the small biases + wob8 + xb (desc-gen on the otherwise-idle
            # Pool engine, transfers interleave behind xt8 on the DMA device).
            xt8_sb = bigx.tile([128, NC_, S], f8)
            xt8_view = xt8_dram[:].rearrange("(c p) s -> p c s", p=128)
            xb_sb = bigx.tile([128, NT, D], bf16)
            xb_view = xb_dram[:].rearrange("(t p) d -> p t d", p=128)
            wqkv8 = consts.tile([128, NC_, 192], f8)
            bqk_row = consts.tile([1, 2, K], f32r)  # [q|k] bias as matmul lhsT
            bv_row8 = consts.tile([1, K], bf16)
            wob8 = consts.tile([33, 2, D], f8)
            # SP/HWDGE side: packed weights then the two xt8 halves
            nc.sync.dma_start(out=wqkv8, in_=wqkv8_dram[:])
            nc.sync.dma_start(out=xt8_sb[:, 0:4, :], in_=xt8_view[:, 0:4, :])
            nc.sync.dma_start(out=xt8_sb[:, 4:8, :], in_=xt8_view[:, 4:8, :])
            # Pool/SWDGE side: biases, wob8, then xb in 8 2-tile chunks
            # (small enough to not hog the DMA device at the avT8 seams)
            nc.gpsimd.dma_start(
                out=bqk_row,
                in_=bqk_dram[:].rearrange("(a j k) -> a j k", a=1, j=2).bitcast(f32r),
            )
            nc.gpsimd.dma_start(
                out=bv_row8, in_=bv_dram[:].rearrange("(a k) -> a k", a=1)
            )
            nc.gpsimd.dma_start(out=wob8, in_=wob8_dram[:])
            for h in range(8):
                nc.gpsimd.dma_start(
                    out=xb_sb[:, 2 * h : 2 * h + 2, :],
                    in_=xb_view[:, 2 * h : 2 * h + 2, :],
                )

            # PE p-state warmup: the cost model halves matmul throughput for
            # the first 3us of any busy streak (and quarters it at streak
            # start). A chain of dependency-free identity matmuls through the
            # xt8-load window ramps PE to full clock before the projections,
            # and keeps the streak alive until xt8's second half lands.
            warm = psS.tile([128, 2, 512], f32, tag="pss", name="warm")
            for _ in range(SCHED["fillers"]):
                nc.tensor.matmul(
                    warm[:, 0, 0:128], identb, identb,
                    start=True, stop=True,
                )

            qk2_sb = proj.tile([K, 2, S], bf16)  # [:,0,:] q, [:,1,:] k
            v_sb = proj.tile([128, NT, 80], f8)  # cols 0:64 v, col 64 = 1/64
            nc.gpsimd.memset(v_sb[:, :, K : K + 1], 1.0 / AVS)
            # avT8 rows 0:32 = uav/256 halves; row 32 ch0 = c-row (sums/32,
            # written per block from crowb), ch1 = 0 so its wob8 row is inert
            avT8 = proj.tile([33, 2, S], f8)
            nc.gpsimd.memset(avT8[32:33, 1, :], 0.0)

            exp_tiles = {}

            def emit_scores(tgt, pair_list):
                if tgt not in exp_tiles:
                    et = expp.tile([128, NT, 512], f8, tag="expT", name=f"expT{tgt}")
                    exp_tiles[tgt] = et
                et = exp_tiles[tgt]
                sqt = slice(tgt * 512, (tgt + 1) * 512)
                for p in pair_list:
                    pss = psS.tile([128, 2, 512], f32, tag="pss", name=f"pss{tgt}_{p}")
                    for j in range(2):
                        sk = 2 * p + j
                        nc.tensor.matmul(
                            pss[:, j, :],
                            qk2_sb[:, 1, sk * 128 : (sk + 1) * 128],
                            qk2_sb[:, 0, sqt],
                            start=True,
                            stop=True,
                        )
                    nc.scalar.activation(
                        out=et[:, 2 * p : 2 * p + 2, :], in_=pss[:],
                        func=AF.Exp, scale=0.125,
                    )

            pair_queue = [(tgt, p) for tgt in range(1, NB) for p in range(NT // 2)]

            def emit_next_pairs(n):
                for _ in range(n):
                    if pair_queue:
                        tgt, p = pair_queue.pop(0)
                        emit_scores(tgt, [p])

            # ---- phase 1: projections; block-0 scores piped in, block-1
            # scores emitted after (so expT(0) completes as early as possible
            # and phase 2 can start under the block-1 exp stream) ----
            with tc.tile_pool(name="psP", bufs=3, space="PSUM") as psP:
                for b in range(NB):
                    sq = slice(b * 512, (b + 1) * 512)
                    # q and k as separate 64-col chains, single-bank tiles
                    # (both land on partitions 0:64; DVE copies cannot cross
                    # partitions); biases folded in as ones-row matmuls.
                    # Block 0 interleaves the q/k chains across the two xt8
                    # DMA halves so PE starts as soon as half 1 lands.
                    psqk = [
                        psP.tile([K, 512], f32, tag=tg, bufs=1, name=f"ps{tg}")
                        for tg in ("psq", "psk")
                    ]
                    for j, cp in [(j, cp) for j in range(2) for cp in range(4)]:
                        nc.tensor.matmul(
                            psqk[j],
                            wqkv8[:, 2 * cp : 2 * cp + 2, j * K : (j + 1) * K],
                            xt8_sb[:, 2 * cp : 2 * cp + 2, sq],
                            start=(cp == 0),
                            stop=False,
                            perf_mode=DR,
                        )
                    for j in range(2):
                        nc.tensor.matmul(
                            psqk[j],
                            bqk_row[:, j, :],
                            ones512r,
                            start=False,
                            stop=True,
                        )
                        if b == 0 and j == 1:
                            # block 0 is latency-critical for the first exp:
                            # run the k copy on the still-idle ACT engine in
                            # parallel with DVE's q copy
                            nc.scalar.activation(
                                out=qk2_sb[:, j, sq], in_=psqk[j],
                                func=AF.Identity,
                            )
                        else:
                            nc.vector.tensor_copy(out=qk2_sb[:, j, sq], in_=psqk[j])
                    # scores for block 0 as its k-tiles become available
                    emit_scores(0, range(b * 2, b * 2 + 2))
                # v projections in a SECOND pass: their 36 matmuls per block
                # otherwise sit between the q/k chains and delay the later
                # blocks' k tiles, which is exactly what starves ACT's early
                # exp stream (tgt-0 pairs are k-availability-bound). v is not
                # needed until attn@v(0) at ~21us.
                for b in range(NB):
                    psv = psP.tile([128, 4, K], f32, tag="psv", bufs=1)
                    for ti in range(4):
                        t = b * 4 + ti
                        for c in range(NC_):
                            nc.tensor.matmul(
                                psv[:, ti, :],
                                xt8_sb[:, c, t * 128 : (t + 1) * 128],
                                wqkv8[:, c, 128:192],
                                start=(c == 0),
                                stop=False,
                            )
                        nc.tensor.matmul(
                            psv[:, ti, :],
                            ones_row128,
                            bv_row8,
                            start=False,
                            stop=True,
                        )
                    nc.vector.tensor_copy(
                        out=v_sb[:, b * 4 : (b + 1) * 4, 0:K], in_=psv
                    )

            # ---- phase 2 ----
            out_view = out_dram[:].rearrange("(t p) d -> p t d", p=128)

            # c-row staging: row 0 carries c = sums/32 per block; rows 1:31
            # stay zero so the per-tile PE transpose of [32,128] chunks reads
            # defined data
            crowb = proj.tile([32, S], f32)
            nc.gpsimd.memset(crowb[:, :], 0.0)

            diags = {}
            psu_by_block = {}

            def emit_block_diags(b):
                # prebuild block b's four diag(c) tiles (transposes into
                # psu_b's spare columns). The per-tile diag otherwise queues
                # behind DVE's stats backlog and gates every y matmul.
                # Called at iteration-(4b) top so the PE park on the c-row
                # cast sits AFTER the previous block's matmuls.
                psu = psu_by_block.pop(b)
                for ti in range(4):
                    t = 4 * b + ti
                    cs = 384 + 32 * ti
                    nc.tensor.transpose(
                        out=psu[:, cs : cs + 32],
                        in_=crowb[:, t * 128 : (t + 1) * 128],
                        identity=ident[0:32, 0:32],
                    )
                    dt_ = work.tile(
                        [128, 128], bf16, tag="diag", name=f"diag{t}"
                    )
                    nc.vector.tensor_scalar(
                        out=dt_, in0=identb, scalar1=psu[:, cs : cs + 1],
                        scalar2=None, op0=mybir.AluOpType.mult,
                    )
                    diags[t] = dt_

            def emit_av(b, splits=1):
                """uav -> av8 = uav/256 cast + c-row = sums/32 for block b,
                then ALL FOUR of the block's diag(c) tiles. Prebuilding the
                diagonals at the seam keeps them out of the per-tile DVE
                queue, whose stats backlog otherwise gates the y matmuls.
                No softmax division: LN's scale-invariance absorbs the
                denominator via the per-row c in the residual diagonal."""
                expT = exp_tiles.pop(b)
                psu = psU.tile([128, 512], f32, tag="psu", name=f"psu{b}")
                for tp in range(NT // 2):
                    nc.tensor.matmul(
                        psu[0 : K + 1, :],
                        v_sb[:, 2 * tp : 2 * tp + 2, 0 : K + 1],
                        expT[:, 2 * tp : 2 * tp + 2, :],
                        start=(tp == 0),
                        stop=(tp == NT // 2 - 1),
                        perf_mode=DR,
                    )
                av8 = avn.tile([K, 512], f8, tag="av8")
                sq = slice(b * 512, (b + 1) * 512)
                with nc.allow_low_precision(reason="uav cast to f8"):
                    nc.vector.tensor_scalar(
                        out=av8, in0=psu[0:K, :], scalar1=1.0 / 256.0,
                        scalar2=None, op0=mybir.AluOpType.mult,
                    )
                nc.vector.tensor_scalar(
                    out=crowb[0:1, sq], in0=psu[K : K + 1, :], scalar1=2.0,
                    scalar2=None, op0=mybir.AluOpType.mult,
                )
                nc.sync.dma_start(out=avT8[0:32, 0, sq], in_=av8[0:32, :])
                nc.sync.dma_start(out=avT8[0:32, 1, sq], in_=av8[32:K, :])
                # c-row into avT8 on Pool (SBUF->SBUF bf16->f8 convert)
                with nc.allow_low_precision(reason="c-row cast to f8"):
                    nc.gpsimd.tensor_copy(
                        out=avT8[32:33, 0, sq], in_=crowb[0:1, sq]
                    )
                psu_by_block[b] = psu
                if taps and b == 0:
                    nc.gpsimd.dma_start(out=tap_handles["T_RECB"][:], in_=crowb[0:8, 0:512])

            if taps:
                nc.gpsimd.dma_start(out=tap_handles["T_QK"][:], in_=qk2_sb[:])
                nc.gpsimd.dma_start(out=tap_handles["T_V"][:], in_=v_sb[:, :, 0 : K + 1])
            # Software-pipelined LN: at iteration t, the j=1 half is normalized
            # on ACT (same engine as rstd, no cross-engine wait); the j=0 half
            # of iteration t-1 is normalized on DVE using the then-ready rstd,
            # so the in-order DVE queue never waits on ACT.
            #
            # Remaining score-pairs (blocks 1-3) are fed from a global queue,
            # 2 per tile iteration AFTER that tile's y-work, so the in-order
            # PE queue never parks y matmuls behind exp-paced score matmuls.
            with tc.tile_pool(name="psY", bufs=3, space="PSUM") as psY:
                if taps:
                    nc.gpsimd.dma_start(
                        out=tap_handles["T_EXP0"][:], in_=exp_tiles[0][:]
                    )
                emit_av(0)
                emit_next_pairs(SCHED["prefill"])  # seam pre-fill
                prev = None  # (psy0, out_sb, mv, rstd, t)
                for b in range(NB):
                    if taps and b == NB - 1:
                        nc.gpsimd.dma_start(out=tap_handles["T_AVT"][:], in_=avT8[:])
                    for ti in range(4):
                        t = b * 4 + ti
                        if ti == 0 and b >= SCHED["diag_pf_from"]:
                            emit_block_diags(b)
                        out_sb = outp.tile([128, D], bf16, tag="o")
                        psy = [None, None]
                        stats = work.tile([128, 2, 6], f32, tag="stats")
                        # block 3: the score-psum pool is free; use its 2-bank
                        # tiles for y so stats/norm run 1024-wide and the LN
                        # pipeline gets extra depth
                        psyt = None
                        if (b == NB - 1 and ti % 2 == 0) or (
                            t == 11 and SCHED["t11_psyt"]
                        ):
                            psyt = psS.tile(
                                [128, 2, 512], f32, tag="pss", name=f"psy2_{t}"
                            )
                            psy[0] = psyt[:, 0, :]
                            psy[1] = psyt[:, 1, :]
                        else:
                            psy[0] = psY.tile([128, 512], f32, tag="ps", name=f"psy0_{t}")
                            psy[1] = psY.tile([128, 512], f32, tag="ps", name=f"psy1_{t}")
                        if t in diags:
                            diag_t = diags.pop(t)
                        else:
                            # per-row diag(c): transpose this tile's c chunk
                            # into spare psy columns (overwritten by the y
                            # matmuls right after diag is built)
                            nc.tensor.transpose(
                                out=psy[0][:, 0:32],
                                in_=crowb[:, t * 128 : (t + 1) * 128],
                                identity=ident[0:32, 0:32],
                            )
                            diag_t = work.tile([128, 128], bf16, tag="diag")
                            nc.vector.tensor_scalar(
                                out=diag_t, in0=identb, scalar1=psy[0][:, 0:1],
                                scalar2=None, op0=mybir.AluOpType.mult,
                            )
                        for j in range(2):
                            psy_j = psy[j]
                            nc.tensor.matmul(
                                psy_j,
                                avT8[:, :, t * 128 : (t + 1) * 128],
                                wob8[:, :, j * 512 : (j + 1) * 512],
                                start=True,
                                stop=False,
                                perf_mode=DR,
                            )
                            nc.tensor.matmul(
                                psy_j,
                                diag_t,
                                xb_sb[:, t, j * 512 : (j + 1) * 512],
                                start=False,
                                stop=True,
                            )
                            nc.vector.bn_stats(out=stats[:, j, :], in_=psy_j)
                        n_emit = 3 if t < SCHED["early3"] else 2
                        if SCHED["emit_pos"] == "mid":
                            emit_next_pairs(n_emit)
                        mv = work.tile([128, 2], f32, tag="mv")
                        nc.vector.bn_aggr(out=mv, in_=stats)
                        # ACT-local chain (no DVE hop): mneg, then
                        # rstd = (var'+epsC)^-0.5 = exp(-0.5*ln(var'+epsC))
                        mneg = work.tile([128, 1], f32, tag="mneg")
                        nc.scalar.mul(mneg, mv[:, 0:1], -1.0)
                        lnv = work.tile([128, 1], f32, tag="lnv")
                        nc.scalar.activation(
                            out=lnv, in_=mv[:, 1:2], func=AF.Ln,
                            bias=epsS_t, scale=1.0,
                        )
                        rstd = work.tile([128, 1], f32, tag="rstd")
                        nc.scalar.activation(
                            out=rstd, in_=lnv, func=AF.Exp, scale=-0.5,
                        )
                        # nm = -mu*rstd, on ACT so the chain stays ACT-local
                        nm = work.tile([128, 1], f32, tag="nm")
                        nc.scalar.activation(
                            out=nm, in_=mneg, func=AF.Copy, scale=rstd,
                        )
                        if psyt is None:
                            nc.scalar.activation(
                                out=out_sb[:, 512:1024], in_=psy[1],
                                func=AF.Identity, bias=nm, scale=rstd,
                            )
                        if prev is not None:
                            p_psy0, p_out, p_mv, p_rstd, p_t = prev
                            nc.vector.tensor_scalar(
                                out=p_out[:, 0:512], in0=p_psy0,
                                scalar1=p_mv[:, 0:1], scalar2=p_rstd,
                                op0=mybir.AluOpType.subtract,
                                op1=mybir.AluOpType.mult,
                            )
                            nc.sync.dma_start(
                                out=out_view[:, p_t, :], in_=p_out
                            )
                            prev = None
                        if psyt is not None:
                            # block-3 + t=11 tiles: exps are done, ACT has
                            # slack and DVE (stats-bound) is critical ->
                            # normalize 1024-wide in one ACT op
                            nc.scalar.activation(
                                out=out_sb[:].rearrange(
                                    "p (j d) -> p j d", j=2
                                ),
                                in_=psyt,
                                func=AF.Identity, bias=nm, scale=rstd,
                            )
                            nc.sync.dma_start(
                                out=out_view[:, t, :], in_=out_sb
                            )
                        elif t >= 12:
                            # non-psyt block-3 tiles: j1 was normalized on ACT
                            # above (stored immediately as its own half so the
                            # DMA overlaps DVE's j0 norm); j0 on DVE here
                            nc.sync.dma_start(
                                out=out_view[:, t, 512:1024],
                                in_=out_sb[:, 512:1024],
                            )
                            nc.vector.tensor_scalar(
                                out=out_sb[:, 0:512], in0=psy[0],
                                scalar1=mv[:, 0:1], scalar2=rstd,
                                op0=mybir.AluOpType.subtract,
                                op1=mybir.AluOpType.mult,
                            )
                            # j0 store from the ACT queue (empty at the
                            # tail), skipping SP's head-of-line descriptor
                            # backlog
                            nc.scalar.dma_start(
                                out=out_view[:, t, 0:512], in_=out_sb[:, 0:512]
                            )
                        elif t >= SCHED["t_act"]:
                            # late non-psyt tiles: exp stream has drained, so
                            # ACT takes the j0 half too (DVE keeps only stats)
                            nc.scalar.activation(
                                out=out_sb[:, 0:512], in_=psy[0],
                                func=AF.Identity, bias=nm, scale=rstd,
                            )
                            nc.sync.dma_start(
                                out=out_view[:, t, :], in_=out_sb
                            )
                        else:
                            prev = (psy[0], out_sb, mv, rstd, t)
                        if ti == (SCHED["av3_ti"] if b == 2 else 2) and b + 1 < NB:
                            emit_av(b + 1)
                        if SCHED["emit_pos"] == "end":
                            emit_next_pairs(n_emit)
                # tail: finish any pending deferred tile
                if prev is not None:
                    p_psy0, p_out, p_mv, p_rstd, p_t = prev
                    nc.vector.tensor_scalar(
                        out=p_out[:, 0:512], in0=p_psy0,
                        scalar1=p_mv[:, 0:1], scalar2=p_rstd,
                        op0=mybir.AluOpType.subtract,
                        op1=mybir.AluOpType.mult,
                    )
                    nc.sync.dma_start(out=out_view[:, p_t, :], in_=p_out)

    nc.compile()
    return nc


def _get_compiled():
    if "nc" not in _COMPILED:
        _COMPILED["nc"] = _build_bass()
    return _COMPILED["nc"]


def _host_inputs(X, Wq, bq, Wk, bk, Wv, bv, Wo, bo):
    import ml_dtypes

    f8 = ml_dtypes.float8_e4m3
    bf = ml_dtypes.bfloat16
    f32 = np.float32

    # [D, 192] = [q|k|v] weights -> [128, 8, 192] with d = c*128 + p
    wqkv = np.concatenate([Wq, Wk, Wv], axis=1).astype(f32)
    wqkv8 = np.ascontiguousarray(
        wqkv.reshape(NC_, 128, 192).transpose(1, 0, 2)
    ).astype(f8)
    bqk = np.concatenate([bq, bk]).astype(f32)
    wob8 = np.zeros((33, 2, D), dtype=f8)
    wo8 = (Wo.astype(f32) * WOS).astype(f8)
    wob8[:32, 0, :] = wo8[0:32]
    wob8[:32, 1, :] = wo8[32:64]
    # row 32 pairs with the c-row (sums/32) in avT8: contributes c*bo
    wob8[32, 0, :] = bo.astype(f32).astype(f8)

    common = {
        "WQKV8": wqkv8,
        "BQK": bqk,
        "BV": bv.astype(f32).astype(bf),
        "WOB8": wob8,
    }
    per_core = []
    for i in range(X.shape[0]):
        Xi = np.ascontiguousarray(X[i], dtype=f32)
        per_core.append(
            {
                "XB": Xi.astype(bf),
                "XT8": np.ascontiguousarray(Xi.T).astype(f8),
                **common,
            }
        )
    return per_core


def kernel(X, Wq, bq, Wk, bk, Wv, bv, Wo, bo, gamma, beta):
    from concourse.bass_utils import run_bass_kernel_spmd

    X = np.asarray(X, dtype=np.float32)
    gamma_np = np.asarray(gamma, dtype=np.float32)
    beta_np = np.asarray(beta, dtype=np.float32)

    nc = _get_compiled()
    in_maps = _host_inputs(
        X,
        np.asarray(Wq), np.asarray(bq), np.asarray(Wk), np.asarray(bk),
        np.asarray(Wv), np.asarray(bv), np.asarray(Wo), np.asarray(bo),
    )
    res = run_bass_kernel_spmd(nc, in_maps, core_ids=list(range(B)))
    out = np.stack(
        [np.asarray(res.results[i]["OUT"]).astype(np.float32) for i in range(B)],
        axis=0,
    )
    if not (np.all(gamma_np == 1.0) and np.all(beta_np == 0.0)):
        out = out * gamma_np + beta_np
    return out.astype(np.float32)

